# revision 1
# baseline (speedup 1.0000x reference)
"""2-layer GAT (PyG semantics) on 8 Trainium2 NeuronCores.

Layout: global degree-sorted node placement, window-interleaved cores.

Node with global-degree-rank r sits in window w = r // 128, partition
p = r % 128; window w belongs to core w % 8 at local window wl = w // 8.
Node table row (in the AllGathered tables) = core * nk + wl * 128 + p.
All per-node tensors (a_dst tables, layer-2 table rows, output rows) are
stored window-major, so every store and every per-node load is a contiguous
DMA; only the per-edge source-row fetch is indirect.
"""
import sys

sys.path.insert(0, '/opt/trn_rl_repo')

from contextlib import ExitStack

import numpy as np

import concourse.bass as bass
import concourse.bacc as bacc
import concourse.mybir as mybir
import concourse.tile as tile
from concourse.masks import make_identity

P = 128
SLOPE = 0.2
D1 = 72          # gather row: [h(64) | a_src(8)]
HEADS = 8
HID = 8
IN_CH = 256

f32 = mybir.dt.float32
bf16 = mybir.dt.bfloat16
i32 = mybir.dt.int32


# ----------------------------------------------------------------------------
def _preprocess(edge_index, n, ncores, colmax=224, wmax=16):
    src = np.asarray(edge_index[0], dtype=np.int64)
    dst = np.asarray(edge_index[1], dtype=np.int64)
    loop = np.arange(n, dtype=np.int64)
    srcs = np.concatenate([src, loop])
    dsts = np.concatenate([dst, loop])

    nk = -(-n // (ncores * P)) * P
    npad = nk * ncores
    nwin = nk // P          # local windows per core
    gwin = nwin * ncores    # global windows

    deg = np.bincount(dsts, minlength=npad).astype(np.int64)
    order = np.argsort(dsts, kind='stable')
    srcs_sorted = srcs[order].astype(np.int64)
    row_ptr = np.zeros(npad + 1, np.int64)
    np.cumsum(deg, out=row_ptr[1:])

    gorder = np.argsort(-deg, kind='stable')        # global degree sort
    # node -> (core, local row): rank r -> w=r//P, p=r%P, core=w%ncores,
    # local row = (w//ncores)*P + p  -> table row = core*nk + local
    r = np.arange(npad)
    w_of = r // P
    core_of = w_of % ncores
    loc_of = (w_of // ncores) * P + (r % P)
    pos = np.empty(npad, np.int64)                  # node -> table row
    pos[gorder] = core_of * nk + loc_of
    nodes_at = np.empty(npad, np.int64)             # table row -> node
    nodes_at[core_of * nk + loc_of] = gorder

    # per local window K (same for all cores at a given wl? no - per core)
    deg_sorted = deg[gorder].reshape(gwin, P)
    k_gwin = deg_sorted.max(axis=1)                 # [gwin] K per global window
    k_loc = k_gwin.reshape(nwin, ncores).T          # [ncores, nwin]
    k_uni = k_loc.max(axis=0)                       # unify across cores
    k_uni = np.maximum(((k_uni + 1) // 2) * 2, 2)

    sws = []
    w = 0
    colstart = 0
    while w < nwin:
        kg = int(k_uni[w])
        nw = 1
        while (w + nw < nwin and nw < wmax
               and (nw + 1) * max(kg, int(k_uni[w + nw])) <= colmax):
            kg = max(kg, int(k_uni[w + nw]))
            nw += 1
        sws.append((w, nw, kg, colstart))
        colstart += nw * kg
        w += nw
    totc = colstart

    offs = np.zeros((ncores, P, totc), np.int32)    # pad slots -> row 0
    mask = np.zeros((ncores, P, totc), np.float32)
    ar = np.arange(P)
    pos32 = pos.astype(np.int32)
    for c in range(ncores):
        for (ws, nw, kg, cs) in sws:
            kar = np.arange(kg)
            for wl in range(nw):
                gids = nodes_at[c * nk + (ws + wl) * P + ar]
                dg = deg[gids]
                st = row_ptr[gids]
                idx = st[:, None] + kar[None, :]
                valid = kar[None, :] < dg[:, None]
                sv = srcs_sorted[np.minimum(idx, len(srcs_sorted) - 1)]
                offs[c, :, cs + wl * kg: cs + (wl + 1) * kg] = \
                    np.where(valid, pos32[sv], 0)
                mask[c, :, cs + wl * kg: cs + (wl + 1) * kg] = valid

    return dict(nk=nk, npad=npad, nwin=nwin, totc=totc, sws=sws,
                offs=offs, mask=mask, nodes_at=nodes_at, pos=pos,
                n_edges=len(srcs_sorted))


# ----------------------------------------------------------------------------
def _build_program(nk, nwin, totc, sws, ncores, in_ch=IN_CH):
    npad = nk * ncores
    nchunk = nk // P
    nhalf = in_ch // P

    nc = bacc.Bacc("TRN2")
    xs = nc.declare_dram_parameter("xs", [nk, in_ch], bf16, isOutput=False)
    w1e = nc.declare_dram_parameter("w1e", [in_ch, 80], bf16, isOutput=False)
    w2e = nc.declare_dram_parameter("w2e", [64, 66], f32, isOutput=False)
    b1r = nc.declare_dram_parameter("b1r", [P, 64], f32, isOutput=False)
    b2r = nc.declare_dram_parameter("b2r", [P, 64], f32, isOutput=False)
    offs = nc.declare_dram_parameter("offs", [P, totc], i32, isOutput=False)
    msk = nc.declare_dram_parameter("msk", [P, totc], bf16, isOutput=False)
    outp = nc.declare_dram_parameter("out", [nk, 64], f32, isOutput=True)

    g1loc = nc.dram_tensor("g1loc", [nk, D1], bf16)
    g2loc = nc.dram_tensor("g2loc", [nk, D1], bf16)
    g1 = nc.dram_tensor("g1", [npad, D1], bf16, addr_space="Shared")
    g2 = nc.dram_tensor("g2", [npad, D1], bf16, addr_space="Shared")
    rg = [list(range(ncores))]

    with ExitStack() as ctx:
        tc = ctx.enter_context(tile.TileContext(nc))
        cp = ctx.enter_context(tc.tile_pool(name="const", bufs=1))
        sb = ctx.enter_context(tc.tile_pool(name="sb", bufs=2))
        sbw = ctx.enter_context(tc.tile_pool(name="sbw", bufs=2))
        sw1 = ctx.enter_context(tc.tile_pool(name="sw1", bufs=1))
        ps = ctx.enter_context(tc.tile_pool(name="ps", bufs=2, space="PSUM"))

        ident = cp.tile([P, P], bf16)
        make_identity(nc, ident[:])
        w1sb = []
        for h in range(nhalf):
            t = cp.tile([P, 80], bf16, tag=f"w1_{h}")
            nc.sync.dma_start(out=t[:], in_=w1e[h * P:(h + 1) * P, :])
            w1sb.append(t)
        w2sb = cp.tile([64, 66], f32)
        nc.sync.dma_start(out=w2sb[:], in_=w2e[:])
        identf = cp.tile([P, P], f32)
        make_identity(nc, identf[:])
        b1sb = cp.tile([P, 64], f32)
        nc.sync.dma_start(out=b1sb[:], in_=b1r[:])
        b2sb = cp.tile([P, 64], f32)
        nc.sync.dma_start(out=b2sb[:], in_=b2r[:])
        ad2all = cp.tile([P, nwin], f32)
        ad1all = cp.tile([P, nwin * 8], f32)

        # ---------------- phase A ----------------
        for cix in range(nchunk):
            xc = sb.tile([P, in_ch], bf16, tag="xc")
            nc.sync.dma_start(out=xc[:], in_=xs[cix * P:(cix + 1) * P, :])
            xts = []
            for h in range(nhalf):
                xt_ps = ps.tile([P, P], bf16, tag="xt")
                nc.tensor.transpose(out=xt_ps[:], in_=xc[:, h * P:(h + 1) * P],
                                    identity=ident[:])
                xt = sb.tile([P, P], bf16, tag=f"xt_{h}")
                nc.vector.tensor_copy(out=xt[:], in_=xt_ps[:])
                xts.append(xt)
            t1_ps = ps.tile([P, 80], f32, tag="t1")
            for h in range(nhalf):
                nc.tensor.matmul(out=t1_ps[:], lhsT=xts[h][:], rhs=w1sb[h][:],
                                 start=(h == 0), stop=(h == nhalf - 1))
            g1row = sb.tile([P, D1], bf16, tag="g1row")
            nc.scalar.copy(out=g1row[:], in_=t1_ps[:, 0:D1])
            nc.vector.tensor_copy(out=ad1all[:, cix * 8:(cix + 1) * 8],
                                  in_=t1_ps[:, D1:80])
            nc.sync.dma_start(out=g1loc[cix * P:(cix + 1) * P, :], in_=g1row[:])

        nc.gpsimd.collective_compute(
            "AllGather", mybir.AluOpType.bypass,
            ins=[g1loc[:]], outs=[g1[:]], replica_groups=rg)

        # ---------------- phase B ----------------
        for (ws, nw, kg, cs) in sws:
            C = nw * kg
            offs_sb = sb.tile([P, C], i32, tag="offs")
            nc.sync.dma_start(out=offs_sb[:], in_=offs[:, cs:cs + C])
            msk_sb = sb.tile([P, C], bf16, tag="msk")
            nc.sync.dma_start(out=msk_sb[:], in_=msk[:, cs:cs + C])

            gb = sbw.tile([P, C * D1], bf16, tag="gb")
            for cj in range(C):
                nc.gpsimd.indirect_dma_start(
                    out=gb[:, cj * D1:(cj + 1) * D1], out_offset=None,
                    in_=g1[:],
                    in_offset=bass.IndirectOffsetOnAxis(
                        ap=offs_sb[:, cj:cj + 1], axis=0))

            gb3 = gb[:].rearrange("p (c d) -> p c d", d=D1)
            alpha = sb.tile([P, C * 8], bf16, tag="alpha")
            a4 = alpha[:].rearrange("p (w k h) -> p w k h", k=kg, h=8)
            nc.vector.tensor_tensor(
                out=a4,
                in0=gb3[:, :, 64:72].rearrange("p (w k) h -> p w k h", k=kg),
                in1=ad1all[:, ws * 8:(ws + nw) * 8]
                    .rearrange("p (w h) -> p w h", h=8)
                    .unsqueeze(2).to_broadcast([P, nw, kg, 8]),
                op=mybir.AluOpType.add)
            lr = sb.tile([P, C * 8], bf16, tag="lr")
            nc.vector.tensor_scalar_mul(out=lr[:], in0=alpha[:], scalar1=SLOPE)
            nc.vector.tensor_tensor(out=lr[:], in0=alpha[:], in1=lr[:],
                                    op=mybir.AluOpType.max)
            u = lr
            nc.scalar.activation(out=u[:], in_=lr[:],
                                 func=mybir.ActivationFunctionType.Exp)
            u3 = u[:].rearrange("p (c h) -> p c h", h=8)
            nc.vector.tensor_tensor(
                out=u3, in0=u3,
                in1=msk_sb[:].unsqueeze(2).to_broadcast([P, C, 8]),
                op=mybir.AluOpType.mult)
            wgh = sw1.tile([P, C * 64], bf16, tag="wgh")
            nc.vector.tensor_tensor(
                out=wgh[:].rearrange("p (c h d) -> p c h d", h=8, d=8),
                in0=gb3[:, :, 0:64].rearrange("p c (h d) -> p c h d", d=8),
                in1=u3.unsqueeze(3).to_broadcast([P, C, 8, 8]),
                op=mybir.AluOpType.mult)
            numer = sb.tile([P, nw * 64], f32, tag="numer")
            nc.vector.tensor_reduce(
                out=numer[:].rearrange("p (w hc) -> p w hc", hc=64),
                in_=wgh[:].rearrange("p (w k hc) -> p w hc k", k=kg, hc=64),
                axis=mybir.AxisListType.X, op=mybir.AluOpType.add)
            denom = sb.tile([P, nw * 8], f32, tag="denom")
            nc.vector.tensor_reduce(
                out=denom[:].rearrange("p (w h) -> p w h", h=8),
                in_=u[:].rearrange("p (w k h) -> p w h k", k=kg, h=8),
                axis=mybir.AxisListType.X, op=mybir.AluOpType.add)
            nc.vector.tensor_scalar_max(out=denom[:], in0=denom[:],
                                        scalar1=1e-30)
            recip = sb.tile([P, nw * 8], f32, tag="recip")
            nc.vector.reciprocal(out=recip[:], in_=denom[:])
            z = sb.tile([P, nw * 64], f32, tag="z")
            z4 = z[:].rearrange("p (w h d) -> p w h d", h=8, d=8)
            nc.vector.tensor_tensor(
                out=z4,
                in0=numer[:].rearrange("p (w h d) -> p w h d", h=8, d=8),
                in1=recip[:].rearrange("p (w h) -> p w h", h=8)
                    .unsqueeze(3).to_broadcast([P, nw, 8, 8]),
                op=mybir.AluOpType.mult)
            z3 = z[:].rearrange("p (w d) -> p w d", d=64)
            nc.vector.tensor_tensor(
                out=z3, in0=z3,
                in1=b1sb[:].unsqueeze(1).to_broadcast([P, nw, 64]),
                op=mybir.AluOpType.add)
            zneg = sb.tile([P, nw * 64], f32, tag="zneg")
            nc.vector.tensor_scalar_min(out=zneg[:], in0=z[:], scalar1=0.0)
            nc.scalar.activation(out=zneg[:], in_=zneg[:],
                                 func=mybir.ActivationFunctionType.Exp)
            nc.vector.tensor_scalar_add(out=zneg[:], in0=zneg[:], scalar1=-1.0)
            nc.vector.tensor_scalar_max(out=z[:], in0=z[:], scalar1=0.0)
            nc.vector.tensor_tensor(out=z[:], in0=z[:], in1=zneg[:],
                                    op=mybir.AluOpType.add)
            g2rows = sb.tile([P, nw * D1], bf16, tag="g2rows")
            nc.vector.memset(g2rows[:], 0.0)
            for wl in range(nw):
                zT_ps = ps.tile([64, P], f32, tag="zt")
                nc.tensor.transpose(out=zT_ps[:],
                                    in_=z[:, wl * 64:(wl + 1) * 64],
                                    identity=identf[:])
                zT = sb.tile([64, P], f32, tag="zts")
                nc.vector.tensor_copy(out=zT[:], in_=zT_ps[:])
                t2_ps = ps.tile([P, 66], f32, tag="t2")
                nc.tensor.matmul(out=t2_ps[:], lhsT=zT[:], rhs=w2sb[:],
                                 start=True, stop=True)
                nc.vector.tensor_copy(out=g2rows[:, wl * D1:wl * D1 + 65],
                                      in_=t2_ps[:, 0:65])
                nc.scalar.copy(out=ad2all[:, ws + wl:ws + wl + 1],
                               in_=t2_ps[:, 65:66])
            nc.sync.dma_start(
                out=g2loc[ws * P:(ws + nw) * P, :]
                    .rearrange("(w p) d -> p w d", p=P),
                in_=g2rows[:].rearrange("p (w d) -> p w d", d=D1))

        nc.gpsimd.collective_compute(
            "AllGather", mybir.AluOpType.bypass,
            ins=[g2loc[:]], outs=[g2[:]], replica_groups=rg)

        # ---------------- phase C ----------------
        for (ws, nw, kg, cs) in sws:
            C = nw * kg
            offs_sb = sb.tile([P, C], i32, tag="offs")
            nc.sync.dma_start(out=offs_sb[:], in_=offs[:, cs:cs + C])
            msk_sb = sb.tile([P, C], bf16, tag="msk")
            nc.sync.dma_start(out=msk_sb[:], in_=msk[:, cs:cs + C])

            gb = sbw.tile([P, C * D1], bf16, tag="gb")
            for cj in range(C):
                nc.gpsimd.indirect_dma_start(
                    out=gb[:, cj * D1:(cj + 1) * D1], out_offset=None,
                    in_=g2[:],
                    in_offset=bass.IndirectOffsetOnAxis(
                        ap=offs_sb[:, cj:cj + 1], axis=0))
            gb3 = gb[:].rearrange("p (c d) -> p c d", d=D1)

            alpha = sb.tile([P, C], f32, tag="alpha2")
            a3 = alpha[:].rearrange("p (w k) -> p w k", k=kg)
            nc.vector.tensor_tensor(
                out=a3,
                in0=gb3[:, :, 64:65].squeeze(2)
                    .rearrange("p (w k) -> p w k", k=kg),
                in1=ad2all[:, ws:ws + nw].unsqueeze(2)
                    .to_broadcast([P, nw, kg]),
                op=mybir.AluOpType.add)
            lr = sb.tile([P, C], f32, tag="lr2")
            nc.vector.tensor_scalar_mul(out=lr[:], in0=alpha[:], scalar1=SLOPE)
            nc.vector.tensor_tensor(out=lr[:], in0=alpha[:], in1=lr[:],
                                    op=mybir.AluOpType.max)
            u = lr
            nc.scalar.activation(out=u[:], in_=lr[:],
                                 func=mybir.ActivationFunctionType.Exp)
            nc.vector.tensor_tensor(out=u[:], in0=u[:], in1=msk_sb[:],
                                    op=mybir.AluOpType.mult)
            wgh = sw1.tile([P, C * 64], bf16, tag="wgh")
            nc.vector.tensor_tensor(
                out=wgh[:].rearrange("p (c d) -> p c d", d=64),
                in0=gb3[:, :, 0:64],
                in1=u[:].unsqueeze(2).to_broadcast([P, C, 64]),
                op=mybir.AluOpType.mult)
            numer = sb.tile([P, nw * 64], f32, tag="numer")
            nc.vector.tensor_reduce(
                out=numer[:].rearrange("p (w d) -> p w d", d=64),
                in_=wgh[:].rearrange("p (w k d) -> p w d k", k=kg, d=64),
                axis=mybir.AxisListType.X, op=mybir.AluOpType.add)
            denom = sb.tile([P, nw], f32, tag="denom2")
            nc.vector.tensor_reduce(
                out=denom[:].unsqueeze(2).squeeze(2),
                in_=u[:].rearrange("p (w k) -> p w k", k=kg),
                axis=mybir.AxisListType.X, op=mybir.AluOpType.add)
            nc.vector.tensor_scalar_max(out=denom[:], in0=denom[:],
                                        scalar1=1e-30)
            recip = sb.tile([P, nw], f32, tag="recip2")
            nc.vector.reciprocal(out=recip[:], in_=denom[:])
            o2 = sb.tile([P, nw * 64], f32, tag="o2")
            o3 = o2[:].rearrange("p (w d) -> p w d", d=64)
            nc.vector.tensor_tensor(
                out=o3,
                in0=numer[:].rearrange("p (w d) -> p w d", d=64),
                in1=recip[:].unsqueeze(2).to_broadcast([P, nw, 64]),
                op=mybir.AluOpType.mult)
            nc.vector.tensor_tensor(
                out=o3, in0=o3,
                in1=b2sb[:].unsqueeze(1).to_broadcast([P, nw, 64]),
                op=mybir.AluOpType.add)
            mx = sb.tile([P, nw], f32, tag="mx")
            nc.vector.tensor_reduce(
                out=mx[:].unsqueeze(2).squeeze(2), in_=o3,
                axis=mybir.AxisListType.X, op=mybir.AluOpType.max)
            nc.vector.tensor_tensor(
                out=o3, in0=o3,
                in1=mx[:].unsqueeze(2).to_broadcast([P, nw, 64]),
                op=mybir.AluOpType.subtract)
            ex = sb.tile([P, nw * 64], f32, tag="ex")
            nc.scalar.activation(out=ex[:], in_=o2[:],
                                 func=mybir.ActivationFunctionType.Exp)
            se = sb.tile([P, nw], f32, tag="se")
            nc.vector.tensor_reduce(
                out=se[:].unsqueeze(2).squeeze(2),
                in_=ex[:].rearrange("p (w d) -> p w d", d=64),
                axis=mybir.AxisListType.X, op=mybir.AluOpType.add)
            nc.scalar.activation(out=se[:], in_=se[:],
                                 func=mybir.ActivationFunctionType.Ln)
            nc.vector.tensor_tensor(
                out=o3, in0=o3,
                in1=se[:].unsqueeze(2).to_broadcast([P, nw, 64]),
                op=mybir.AluOpType.subtract)
            nc.sync.dma_start(
                out=outp[ws * P:(ws + nw) * P, :]
                    .rearrange("(w p) d -> p w d", p=P),
                in_=o2[:].rearrange("p (w d) -> p w d", d=64))
    nc.compile()
    return nc


_NO_SPLIT = (mybir.InstEventSemaphore,)


def _split_dma_waits(nc, maxw=1):
    """Walrus encodes at most one sync wait per engine instruction; offload
    the excess onto standalone sequencer waits placed just before it."""
    dummy = nc._xw_dummy
    for bb in nc.m.functions[0].blocks:
        new = []
        for ins in bb.instructions:
            si = getattr(ins, 'sync_info', None)
            if (not isinstance(ins, _NO_SPLIT)
                    and getattr(ins, 'engine', None) is not None
                    and si is not None
                    and si.on_wait and len(si.on_wait) > maxw):
                waits = list(si.on_wait)
                for i, w in enumerate(waits[:-maxw]):
                    nop_upd = mybir.SyncUpdate(
                        sync_type='semaphore', id=dummy.num,
                        ant_name=dummy.name,
                        update_mode='sem-inc', update_value=1)
                    wi = mybir.InstEventSemaphore(
                        name=f"{ins.name}-xw{i}", engine=ins.engine,
                        ins=[], outs=[],
                        sync_info=mybir.SyncInfo(on_wait=[w],
                                                 on_update=[nop_upd]))
                    new.append(wi)
                si.on_wait = waits[-maxw:]
            new.append(ins)
        bb.instructions = new


# ----------------------------------------------------------------------------
def _expand_weights(W1, att_src1, att_dst1, W2, att_src2, att_dst2):
    W1 = np.asarray(W1, np.float32)
    a1s = np.zeros((HEADS * HID, HEADS), np.float32)
    a1s[np.arange(HEADS * HID), np.arange(HEADS * HID) // HID] = \
        np.asarray(att_src1, np.float32).reshape(-1)
    a1d = np.zeros((HEADS * HID, HEADS), np.float32)
    a1d[np.arange(HEADS * HID), np.arange(HEADS * HID) // HID] = \
        np.asarray(att_dst1, np.float32).reshape(-1)
    w1e = np.concatenate([W1, W1 @ a1s, W1 @ a1d], axis=1)
    W2 = np.asarray(W2, np.float32)
    w2e = np.concatenate(
        [W2,
         W2 @ np.asarray(att_src2, np.float32).T,
         W2 @ np.asarray(att_dst2, np.float32).T], axis=1)
    return np.ascontiguousarray(w1e), np.ascontiguousarray(w2e)


def _make_in_maps(pre, x, w1e, w2e, b1, b2, ncores):
    import ml_dtypes
    nk = pre['nk']
    n = x.shape[0]
    npad = pre['npad']
    xpad = np.zeros((npad, x.shape[1]), np.float32)
    xpad[:n] = np.asarray(x, np.float32)
    xperm = xpad[pre['nodes_at']]                  # table-row order
    xperm_bf = xperm.astype(ml_dtypes.bfloat16)
    w1e_bf = w1e.astype(ml_dtypes.bfloat16)
    b1r = np.broadcast_to(np.asarray(b1, np.float32)[None, :], (P, 64)).copy()
    b2r = np.broadcast_to(np.asarray(b2, np.float32)[None, :], (P, 64)).copy()
    in_maps = []
    for c in range(ncores):
        in_maps.append(dict(
            xs=np.ascontiguousarray(xperm_bf[c * nk:(c + 1) * nk]),
            w1e=w1e_bf, w2e=w2e, b1r=b1r, b2r=b2r,
            offs=np.ascontiguousarray(pre['offs'][c]),
            msk=np.ascontiguousarray(
                pre['mask'][c].astype(ml_dtypes.bfloat16)),
        ))
    return in_maps


def _postprocess(res, pre, n, ncores):
    out = np.concatenate([res[c]["out"] for c in range(ncores)], axis=0)
    return np.ascontiguousarray(out[pre['pos'][:n]]).astype(np.float32)


def kernel(x, edge_index, W1, att_src1, att_dst1, b1,
           W2, att_src2, att_dst2, b2):
    import os
    os.environ["BASS_NEVER_TRACE"] = "1"
    from concourse.bass_utils import run_bass_kernel_spmd
    ncores = 8
    n = x.shape[0]
    pre = _preprocess(np.asarray(edge_index), n, ncores)
    w1e, w2e = _expand_weights(W1, att_src1, att_dst1, W2, att_src2, att_dst2)
    in_maps = _make_in_maps(pre, x, w1e, w2e, b1, b2, ncores)
    nc = _build_program(pre['nk'], pre['nwin'], pre['totc'], pre['sws'],
                        ncores, in_ch=x.shape[1])
    res = run_bass_kernel_spmd(nc, in_maps, list(range(ncores))).results
    return _postprocess(res, pre, n, ncores)



# revision 18
# speedup vs baseline: 1.1213x; 1.1213x over previous
"""2-layer GAT (PyG semantics) on 8 Trainium2 NeuronCores.

Layout: global degree-sorted node placement, window-interleaved cores.

Node with global-degree-rank r sits in window w = r // 128, partition
p = r % 128; window w belongs to core w % 8 at local window wl = w // 8.
Node table row (in the AllGathered tables) = core * nk + wl * 128 + p.
All per-node tensors (a_dst tables, layer-2 table rows, output rows) are
stored window-major, so every store and every per-node load is a contiguous
DMA; only the per-edge source-row fetch is indirect.

k-slot 0 of every window is the self-loop: its source row is the node
itself, which lives at an affine position in the core's local table slice,
so it is loaded with one dense DMA per group instead of indirect gathers.
"""
import sys

sys.path.insert(0, '/opt/trn_rl_repo')

from contextlib import ExitStack

import numpy as np

import concourse.bass as bass
import concourse.bacc as bacc
import concourse.mybir as mybir
import concourse.tile as tile
from concourse.masks import make_identity

P = 128
SLOPE = 0.2
D1 = 72          # gather row: [h(64) | a_src(8)]
HEADS = 8
HID = 8
IN_CH = 256

f32 = mybir.dt.float32
bf16 = mybir.dt.bfloat16
i32 = mybir.dt.int32


# ----------------------------------------------------------------------------
def _preprocess(edge_index, n, ncores, colmax=224, wmax=16):
    src = np.asarray(edge_index[0], dtype=np.int64)
    dst = np.asarray(edge_index[1], dtype=np.int64)

    nk = -(-n // (ncores * P)) * P
    npad = nk * ncores
    nwin = nk // P          # local windows per core
    gwin = nwin * ncores    # global windows

    deg = np.bincount(dst, minlength=npad).astype(np.int64)  # real edges only
    order = np.argsort(dst, kind='stable')
    srcs_sorted = src[order].astype(np.int64)
    row_ptr = np.zeros(npad + 1, np.int64)
    np.cumsum(deg, out=row_ptr[1:])

    gorder = np.argsort(-deg, kind='stable')        # global degree sort
    r = np.arange(npad)
    w_of = r // P
    core_of = w_of % ncores
    loc_of = (w_of // ncores) * P + (r % P)
    pos = np.empty(npad, np.int64)                  # node -> local row
    pos[gorder] = core_of * nk + loc_of
    nodes_at = np.empty(npad, np.int64)             # local row -> node
    nodes_at[core_of * nk + loc_of] = gorder

    # shared-table layout is chunk-major so AllGather chunks are contiguous:
    # row = chunk_base*ncores + core*chunk_size + (loc - chunk_start)
    sup = max(s for s in range(1, 9) if nwin % s == 0)
    cb = (nwin // sup // 2) * sup * P               # chunk boundary (rows)
    in_c1 = loc_of >= cb
    pos_tab = np.where(
        in_c1,
        cb * ncores + core_of * (nk - cb) + (loc_of - cb),
        core_of * cb + loc_of)
    postab = np.empty(npad, np.int64)               # node -> shared-table row
    postab[gorder] = pos_tab

    deg_sorted = deg[gorder].reshape(gwin, P)
    k_gwin = deg_sorted.max(axis=1)                 # data-K per global window
    k_loc = k_gwin.reshape(nwin, ncores).T
    k_uni = k_loc.max(axis=0) + 1                   # +1 self column (slot 0)
    k_uni = np.maximum(k_uni, 2)

    sws = []
    w = 0
    colstart = 0
    while w < nwin:
        kg = int(k_uni[w])
        nw = 1
        while (w + nw < nwin and nw < wmax
               and (nw + 1) * max(kg, int(k_uni[w + nw])) <= colmax):
            kg = max(kg, int(k_uni[w + nw]))
            nw += 1
        sws.append((w, nw, kg, colstart))
        colstart += nw * kg
        w += nw
    totc = colstart

    offs = np.zeros((ncores, P, totc), np.int32)    # pad slots -> row 0
    mask = np.zeros((ncores, P, totc), np.float32)
    ar = np.arange(P)
    pos32 = postab.astype(np.int32)
    for c in range(ncores):
        for (ws, nw, kg, cs) in sws:
            kar = np.arange(kg - 1)
            for wl in range(nw):
                rows = c * nk + (ws + wl) * P + ar
                gids = nodes_at[rows]
                dg = deg[gids]
                st = row_ptr[gids]
                idx = st[:, None] + kar[None, :]
                valid = kar[None, :] < dg[:, None]
                sv = srcs_sorted[np.minimum(idx, max(len(srcs_sorted) - 1, 0))]
                c0 = cs + wl * kg
                offs[c, :, c0] = rows                       # self (not gathered)
                mask[c, :, c0] = 1.0
                offs[c, :, c0 + 1: c0 + kg] = np.where(valid, pos32[sv], 0)
                mask[c, :, c0 + 1: c0 + kg] = valid

    return dict(nk=nk, npad=npad, nwin=nwin, totc=totc, sws=sws,
                offs=offs, mask=mask, nodes_at=nodes_at, pos=pos,
                n_edges=len(srcs_sorted))


# ----------------------------------------------------------------------------
def _build_program(nk, nwin, totc, sws, ncores, in_ch=IN_CH, reps=1):
    npad = nk * ncores
    nhalf = in_ch // P

    nc = bacc.Bacc("TRN2")
    xt = nc.declare_dram_parameter("xt", [in_ch, nk], bf16, isOutput=False)
    w1e = nc.declare_dram_parameter("w1e", [in_ch, 80], bf16, isOutput=False)
    w2e = nc.declare_dram_parameter("w2e", [64, 66], f32, isOutput=False)
    b1r = nc.declare_dram_parameter("b1r", [P, 64], f32, isOutput=False)
    b2r = nc.declare_dram_parameter("b2r", [P, 64], f32, isOutput=False)
    offs = nc.declare_dram_parameter("offs", [P, totc], i32, isOutput=False)
    msk = nc.declare_dram_parameter("msk", [P, totc], bf16, isOutput=False)
    outp = nc.declare_dram_parameter("out", [nk, 64], f32, isOutput=True)

    g1loc = nc.dram_tensor("g1loc", [nk, D1], bf16)
    g2loc = nc.dram_tensor("g2loc", [nk, D1], bf16)
    g1 = nc.dram_tensor("g1", [npad, D1], bf16, addr_space="Shared")
    g2 = nc.dram_tensor("g2", [npad, D1], bf16, addr_space="Shared")
    rg = [list(range(ncores))]

    SUP = max(s for s in range(1, 9) if nwin % s == 0)
    nsup = nwin // SUP
    cb = (nsup // 2) * SUP * P          # chunk boundary (matches _preprocess)

    with ExitStack() as ctx:
        tc = ctx.enter_context(tile.TileContext(nc))
        cp = ctx.enter_context(tc.tile_pool(name="const", bufs=1))
        sa = ctx.enter_context(tc.tile_pool(name="sa", bufs=2))
        sb = ctx.enter_context(tc.tile_pool(name="sb", bufs=2))
        sbw = ctx.enter_context(tc.tile_pool(name="sbw", bufs=2))
        sw1 = ctx.enter_context(tc.tile_pool(name="sw1", bufs=1))
        ps = ctx.enter_context(tc.tile_pool(name="ps", bufs=2, space="PSUM"))

        w1sb = []
        for h in range(nhalf):
            t = cp.tile([P, 80], bf16, tag=f"w1_{h}")
            nc.sync.dma_start(out=t[:], in_=w1e[h * P:(h + 1) * P, :])
            w1sb.append(t)
        w2sb = cp.tile([64, 66], f32)
        nc.sync.dma_start(out=w2sb[:], in_=w2e[:])
        identf = cp.tile([P, P], f32)
        make_identity(nc, identf[:])
        b1sb = cp.tile([P, 64], f32)
        nc.sync.dma_start(out=b1sb[:], in_=b1r[:])
        b2sb = cp.tile([P, 64], f32)
        nc.sync.dma_start(out=b2sb[:], in_=b2r[:])
        ad2all = cp.tile([P, nwin], f32)
        ad1all = cp.tile([P, nwin * 8], f32)
        offs_sb = cp.tile([P, totc], i32)
        nc.sync.dma_start(out=offs_sb[:], in_=offs[:])
        msk_sb = cp.tile([P, totc], bf16)
        nc.sync.dma_start(out=msk_sb[:], in_=msk[:])

        for _ in range(reps):
            # ---------------- phase A ----------------
            for sc in range(nsup):
                xts = []
                for h in range(nhalf):
                    t = sa.tile([P, SUP * P], bf16, tag=f"xt_{h}")
                    nc.sync.dma_start(
                        out=t[:],
                        in_=xt[h * P:(h + 1) * P,
                               sc * SUP * P:(sc + 1) * SUP * P])
                    xts.append(t)
                g1rows = sa.tile([P, SUP * D1], bf16, tag="g1rows")
                for wl in range(SUP):
                    t1_ps = ps.tile([P, 80], f32, tag="t1")
                    for h in range(nhalf):
                        nc.tensor.matmul(
                            out=t1_ps[:],
                            lhsT=xts[h][:, wl * P:(wl + 1) * P],
                            rhs=w1sb[h][:],
                            start=(h == 0), stop=(h == nhalf - 1))
                    nc.scalar.copy(out=g1rows[:, wl * D1:(wl + 1) * D1],
                                   in_=t1_ps[:, 0:D1])
                    cix = sc * SUP + wl
                    nc.vector.tensor_copy(
                        out=ad1all[:, cix * 8:(cix + 1) * 8],
                        in_=t1_ps[:, D1:80])
                nc.sync.dma_start(
                    out=g1loc[sc * SUP * P:(sc + 1) * SUP * P, :]
                        .rearrange("(w p) d -> p w d", p=P),
                    in_=g1rows[:].rearrange("p (w d) -> p w d", d=D1))
                if sc == nsup // 2 - 1:
                    nc.gpsimd.collective_compute(
                        "AllGather", mybir.AluOpType.bypass,
                        ins=[g1loc[0:cb]],
                        outs=[g1[0:cb * ncores]],
                        replica_groups=rg)
            nc.gpsimd.collective_compute(
                "AllGather", mybir.AluOpType.bypass,
                ins=[g1loc[cb:nk]],
                outs=[g1[cb * ncores:npad]],
                replica_groups=rg)

            # ---------------- phase B ----------------
            nb2 = 0
            for gi, (ws, nw, kg, cs) in enumerate(sws):
                C = nw * kg
                gb = sbw.tile([P, C * D1], bf16, tag="gb")
                # self columns (k=0 of each window): one dense DMA
                nc.sync.dma_start(
                    out=gb[:].rearrange("p (w k d) -> p w k d", k=kg, d=D1)[
                        :, :, 0:1, :].squeeze(2),
                    in_=g1loc[ws * P:(ws + nw) * P, :]
                        .rearrange("(w p) d -> p w d", p=P))
                for cj in range(C):
                    if cj % kg == 0:
                        continue
                    nc.gpsimd.indirect_dma_start(
                        out=gb[:, cj * D1:(cj + 1) * D1], out_offset=None,
                        in_=g1[:],
                        in_offset=bass.IndirectOffsetOnAxis(
                            ap=offs_sb[:, cs + cj:cs + cj + 1], axis=0))

                gb3 = gb[:].rearrange("p (c d) -> p c d", d=D1)
                alpha = sb.tile([P, C * 8], bf16, tag="alpha")
                a4 = alpha[:].rearrange("p (w k h) -> p w k h", k=kg, h=8)
                nc.vector.tensor_tensor(
                    out=a4,
                    in0=gb3[:, :, 64:72].rearrange("p (w k) h -> p w k h", k=kg),
                    in1=ad1all[:, ws * 8:(ws + nw) * 8]
                        .rearrange("p (w h) -> p w h", h=8)
                        .unsqueeze(2).to_broadcast([P, nw, kg, 8]),
                    op=mybir.AluOpType.add)
                lr = sb.tile([P, C * 8], bf16, tag="lr")
                nc.vector.tensor_scalar_mul(out=lr[:], in0=alpha[:],
                                            scalar1=SLOPE)
                nc.vector.tensor_tensor(out=lr[:], in0=alpha[:], in1=lr[:],
                                        op=mybir.AluOpType.max)
                u = lr
                nc.scalar.activation(out=u[:], in_=lr[:],
                                     func=mybir.ActivationFunctionType.Exp)
                u3 = u[:].rearrange("p (c h) -> p c h", h=8)
                nc.vector.tensor_tensor(
                    out=u3, in0=u3,
                    in1=msk_sb[:, cs:cs + C].unsqueeze(2).to_broadcast(
                        [P, C, 8]),
                    op=mybir.AluOpType.mult)
                wgh = sw1.tile([P, C * 64], bf16, tag="wgh")
                nc.vector.tensor_tensor(
                    out=wgh[:].rearrange("p (c h d) -> p c h d", h=8, d=8),
                    in0=gb3[:, :, 0:64].rearrange("p c (h d) -> p c h d", d=8),
                    in1=u3.unsqueeze(3).to_broadcast([P, C, 8, 8]),
                    op=mybir.AluOpType.mult)
                numer = sb.tile([P, nw * 64], f32, tag="numer")
                nc.vector.tensor_reduce(
                    out=numer[:].rearrange("p (w hc) -> p w hc", hc=64),
                    in_=wgh[:].rearrange("p (w k hc) -> p w hc k", k=kg, hc=64),
                    axis=mybir.AxisListType.X, op=mybir.AluOpType.add)
                denom = sb.tile([P, nw * 8], f32, tag="denom")
                nc.vector.tensor_reduce(
                    out=denom[:].rearrange("p (w h) -> p w h", h=8),
                    in_=u[:].rearrange("p (w k h) -> p w h k", k=kg, h=8),
                    axis=mybir.AxisListType.X, op=mybir.AluOpType.add)
                nc.vector.tensor_scalar_max(out=denom[:], in0=denom[:],
                                            scalar1=1e-30)
                recip = sb.tile([P, nw * 8], f32, tag="recip")
                nc.vector.reciprocal(out=recip[:], in_=denom[:])
                z = sb.tile([P, nw * 64], f32, tag="z")
                z4 = z[:].rearrange("p (w h d) -> p w h d", h=8, d=8)
                nc.vector.tensor_tensor(
                    out=z4,
                    in0=numer[:].rearrange("p (w h d) -> p w h d", h=8, d=8),
                    in1=recip[:].rearrange("p (w h) -> p w h", h=8)
                        .unsqueeze(3).to_broadcast([P, nw, 8, 8]),
                    op=mybir.AluOpType.mult)
                z3 = z[:].rearrange("p (w d) -> p w d", d=64)
                nc.vector.tensor_tensor(
                    out=z3, in0=z3,
                    in1=b1sb[:].unsqueeze(1).to_broadcast([P, nw, 64]),
                    op=mybir.AluOpType.add)
                zneg = sb.tile([P, nw * 64], f32, tag="zneg")
                nc.vector.tensor_scalar_min(out=zneg[:], in0=z[:], scalar1=0.0)
                nc.scalar.activation(out=zneg[:], in_=zneg[:],
                                     func=mybir.ActivationFunctionType.Exp)
                nc.vector.tensor_scalar_add(out=zneg[:], in0=zneg[:],
                                            scalar1=-1.0)
                nc.vector.tensor_scalar_max(out=z[:], in0=z[:], scalar1=0.0)
                nc.vector.tensor_tensor(out=z[:], in0=z[:], in1=zneg[:],
                                        op=mybir.AluOpType.add)
                g2rows = sb.tile([P, nw * D1], bf16, tag="g2rows")
                nc.vector.memset(g2rows[:], 0.0)
                for wl in range(nw):
                    zT_ps = ps.tile([64, P], f32, tag="zt")
                    nc.tensor.transpose(out=zT_ps[:],
                                        in_=z[:, wl * 64:(wl + 1) * 64],
                                        identity=identf[:])
                    zT = sb.tile([64, P], f32, tag="zts")
                    nc.vector.tensor_copy(out=zT[:], in_=zT_ps[:])
                    t2_ps = ps.tile([P, 66], f32, tag="t2")
                    nc.tensor.matmul(out=t2_ps[:], lhsT=zT[:], rhs=w2sb[:],
                                     start=True, stop=True)
                    nc.vector.tensor_copy(
                        out=g2rows[:, wl * D1:wl * D1 + 65],
                        in_=t2_ps[:, 0:65])
                    nc.scalar.copy(out=ad2all[:, ws + wl:ws + wl + 1],
                                   in_=t2_ps[:, 65:66])
                nc.sync.dma_start(
                    out=g2loc[ws * P:(ws + nw) * P, :]
                        .rearrange("(w p) d -> p w d", p=P),
                    in_=g2rows[:].rearrange("p (w d) -> p w d", d=D1))
                # AllGather g2 in the same 2 chunks as g1 (contiguous outs)
                if nb2 == 0 and (ws + nw) * P >= cb:
                    nc.gpsimd.collective_compute(
                        "AllGather", mybir.AluOpType.bypass,
                        ins=[g2loc[0:cb]],
                        outs=[g2[0:cb * ncores]],
                        replica_groups=rg)
                    nb2 = cb
                elif gi == len(sws) - 1:
                    nc.gpsimd.collective_compute(
                        "AllGather", mybir.AluOpType.bypass,
                        ins=[g2loc[cb:nk]],
                        outs=[g2[cb * ncores:npad]],
                        replica_groups=rg)

            # ---------------- phase C ----------------
            for (ws, nw, kg, cs) in sws:
                C = nw * kg
                gb = sbw.tile([P, C * D1], bf16, tag="gb")
                nc.sync.dma_start(
                    out=gb[:].rearrange("p (w k d) -> p w k d", k=kg, d=D1)[
                        :, :, 0:1, :].squeeze(2),
                    in_=g2loc[ws * P:(ws + nw) * P, :]
                        .rearrange("(w p) d -> p w d", p=P))
                for cj in range(C):
                    if cj % kg == 0:
                        continue
                    nc.gpsimd.indirect_dma_start(
                        out=gb[:, cj * D1:(cj + 1) * D1], out_offset=None,
                        in_=g2[:],
                        in_offset=bass.IndirectOffsetOnAxis(
                            ap=offs_sb[:, cs + cj:cs + cj + 1], axis=0))
                gb3 = gb[:].rearrange("p (c d) -> p c d", d=D1)

                alpha = sb.tile([P, C], f32, tag="alpha2")
                a3 = alpha[:].rearrange("p (w k) -> p w k", k=kg)
                nc.vector.tensor_tensor(
                    out=a3,
                    in0=gb3[:, :, 64:65].squeeze(2)
                        .rearrange("p (w k) -> p w k", k=kg),
                    in1=ad2all[:, ws:ws + nw].unsqueeze(2)
                        .to_broadcast([P, nw, kg]),
                    op=mybir.AluOpType.add)
                lr = sb.tile([P, C], f32, tag="lr2")
                nc.vector.tensor_scalar_mul(out=lr[:], in0=alpha[:],
                                            scalar1=SLOPE)
                nc.vector.tensor_tensor(out=lr[:], in0=alpha[:], in1=lr[:],
                                        op=mybir.AluOpType.max)
                u = lr
                nc.scalar.activation(out=u[:], in_=lr[:],
                                     func=mybir.ActivationFunctionType.Exp)
                nc.vector.tensor_tensor(out=u[:], in0=u[:],
                                        in1=msk_sb[:, cs:cs + C],
                                        op=mybir.AluOpType.mult)
                wgh = sw1.tile([P, C * 64], bf16, tag="wgh")
                nc.vector.tensor_tensor(
                    out=wgh[:].rearrange("p (c d) -> p c d", d=64),
                    in0=gb3[:, :, 0:64],
                    in1=u[:].unsqueeze(2).to_broadcast([P, C, 64]),
                    op=mybir.AluOpType.mult)
                numer = sb.tile([P, nw * 64], f32, tag="numer")
                nc.vector.tensor_reduce(
                    out=numer[:].rearrange("p (w d) -> p w d", d=64),
                    in_=wgh[:].rearrange("p (w k d) -> p w d k", k=kg, d=64),
                    axis=mybir.AxisListType.X, op=mybir.AluOpType.add)
                denom = sb.tile([P, nw], f32, tag="denom2")
                nc.vector.tensor_reduce(
                    out=denom[:].unsqueeze(2).squeeze(2),
                    in_=u[:].rearrange("p (w k) -> p w k", k=kg),
                    axis=mybir.AxisListType.X, op=mybir.AluOpType.add)
                nc.vector.tensor_scalar_max(out=denom[:], in0=denom[:],
                                            scalar1=1e-30)
                recip = sb.tile([P, nw], f32, tag="recip2")
                nc.vector.reciprocal(out=recip[:], in_=denom[:])
                o2 = sb.tile([P, nw * 64], f32, tag="o2")
                o3 = o2[:].rearrange("p (w d) -> p w d", d=64)
                nc.vector.tensor_tensor(
                    out=o3,
                    in0=numer[:].rearrange("p (w d) -> p w d", d=64),
                    in1=recip[:].unsqueeze(2).to_broadcast([P, nw, 64]),
                    op=mybir.AluOpType.mult)
                nc.vector.tensor_tensor(
                    out=o3, in0=o3,
                    in1=b2sb[:].unsqueeze(1).to_broadcast([P, nw, 64]),
                    op=mybir.AluOpType.add)
                mx = sb.tile([P, nw], f32, tag="mx")
                nc.vector.tensor_reduce(
                    out=mx[:].unsqueeze(2).squeeze(2), in_=o3,
                    axis=mybir.AxisListType.X, op=mybir.AluOpType.max)
                nc.vector.tensor_tensor(
                    out=o3, in0=o3,
                    in1=mx[:].unsqueeze(2).to_broadcast([P, nw, 64]),
                    op=mybir.AluOpType.subtract)
                ex = sb.tile([P, nw * 64], f32, tag="ex")
                nc.scalar.activation(out=ex[:], in_=o2[:],
                                     func=mybir.ActivationFunctionType.Exp)
                se = sb.tile([P, nw], f32, tag="se")
                nc.vector.tensor_reduce(
                    out=se[:].unsqueeze(2).squeeze(2),
                    in_=ex[:].rearrange("p (w d) -> p w d", d=64),
                    axis=mybir.AxisListType.X, op=mybir.AluOpType.add)
                nc.scalar.activation(out=se[:], in_=se[:],
                                     func=mybir.ActivationFunctionType.Ln)
                nc.vector.tensor_tensor(
                    out=o3, in0=o3,
                    in1=se[:].unsqueeze(2).to_broadcast([P, nw, 64]),
                    op=mybir.AluOpType.subtract)
                nc.sync.dma_start(
                    out=outp[ws * P:(ws + nw) * P, :]
                        .rearrange("(w p) d -> p w d", p=P),
                    in_=o2[:].rearrange("p (w d) -> p w d", d=64))
    nc.compile()
    return nc


# ----------------------------------------------------------------------------
def _expand_weights(W1, att_src1, att_dst1, W2, att_src2, att_dst2):
    W1 = np.asarray(W1, np.float32)
    a1s = np.zeros((HEADS * HID, HEADS), np.float32)
    a1s[np.arange(HEADS * HID), np.arange(HEADS * HID) // HID] = \
        np.asarray(att_src1, np.float32).reshape(-1)
    a1d = np.zeros((HEADS * HID, HEADS), np.float32)
    a1d[np.arange(HEADS * HID), np.arange(HEADS * HID) // HID] = \
        np.asarray(att_dst1, np.float32).reshape(-1)
    w1e = np.concatenate([W1, W1 @ a1s, W1 @ a1d], axis=1)
    W2 = np.asarray(W2, np.float32)
    w2e = np.concatenate(
        [W2,
         W2 @ np.asarray(att_src2, np.float32).T,
         W2 @ np.asarray(att_dst2, np.float32).T], axis=1)
    return np.ascontiguousarray(w1e), np.ascontiguousarray(w2e)


def _make_in_maps(pre, x, w1e, w2e, b1, b2, ncores):
    import ml_dtypes
    nk = pre['nk']
    n = x.shape[0]
    npad = pre['npad']
    xpad = np.zeros((npad, x.shape[1]), np.float32)
    xpad[:n] = np.asarray(x, np.float32)
    xperm = xpad[pre['nodes_at']]                  # table-row order
    w1e_bf = w1e.astype(ml_dtypes.bfloat16)
    b1r = np.broadcast_to(np.asarray(b1, np.float32)[None, :], (P, 64)).copy()
    b2r = np.broadcast_to(np.asarray(b2, np.float32)[None, :], (P, 64)).copy()
    in_maps = []
    for c in range(ncores):
        xtc = np.ascontiguousarray(
            xperm[c * nk:(c + 1) * nk].T).astype(ml_dtypes.bfloat16)
        in_maps.append(dict(
            xt=xtc,
            w1e=w1e_bf, w2e=w2e, b1r=b1r, b2r=b2r,
            offs=np.ascontiguousarray(pre['offs'][c]),
            msk=np.ascontiguousarray(
                pre['mask'][c].astype(ml_dtypes.bfloat16)),
        ))
    return in_maps


def _postprocess(res, pre, n, ncores):
    out = np.concatenate([res[c]["out"] for c in range(ncores)], axis=0)
    return np.ascontiguousarray(out[pre['pos'][:n]]).astype(np.float32)


def kernel(x, edge_index, W1, att_src1, att_dst1, b1,
           W2, att_src2, att_dst2, b2):
    import os
    os.environ["BASS_NEVER_TRACE"] = "1"
    from concourse.bass_utils import run_bass_kernel_spmd
    ncores = 8
    n = x.shape[0]
    pre = _preprocess(np.asarray(edge_index), n, ncores)
    w1e, w2e = _expand_weights(W1, att_src1, att_dst1, W2, att_src2, att_dst2)
    in_maps = _make_in_maps(pre, x, w1e, w2e, b1, b2, ncores)
    nc = _build_program(pre['nk'], pre['nwin'], pre['totc'], pre['sws'],
                        ncores, in_ch=x.shape[1])
    res = run_bass_kernel_spmd(nc, in_maps, list(range(ncores))).results
    return _postprocess(res, pre, n, ncores)


# revision 29
# speedup vs baseline: 1.2731x; 1.1354x over previous
"""2-layer GAT (PyG semantics) on 8 Trainium2 NeuronCores.

Layout: global degree-sorted node placement, window-interleaved cores.

Node with global-degree-rank r sits in window w = r // 128, partition
p = r % 128; window w belongs to core w % 8 at local window wl = w // 8.
Node table row (in the AllGathered tables) = core * nk + wl * 128 + p.
All per-node tensors (a_dst tables, layer-2 table rows, output rows) are
stored window-major, so every store and every per-node load is a contiguous
DMA; only the per-edge source-row fetch is indirect.

k-slot 0 of every window is the self-loop: its source row is the node
itself, which lives at an affine position in the core's local table slice,
so it is loaded with one dense DMA per group instead of indirect gathers.
"""
import sys

sys.path.insert(0, '/opt/trn_rl_repo')

from contextlib import ExitStack

import numpy as np

import concourse.bass as bass
import concourse.bacc as bacc
import concourse.mybir as mybir
import concourse.tile as tile
from concourse.masks import make_identity

P = 128
SLOPE = 0.2
D1 = 72          # gather row: [h(64) | a_src(8)]
HEADS = 8
HID = 8
IN_CH = 256

f32 = mybir.dt.float32
bf16 = mybir.dt.bfloat16
i32 = mybir.dt.int32


# ----------------------------------------------------------------------------
def _preprocess(edge_index, n, ncores, colmax=64, wmax=4):
    src = np.asarray(edge_index[0], dtype=np.int64)
    dst = np.asarray(edge_index[1], dtype=np.int64)

    nk = -(-n // (ncores * P)) * P
    npad = nk * ncores
    nwin = nk // P          # local windows per core
    gwin = nwin * ncores    # global windows

    deg = np.bincount(dst, minlength=npad).astype(np.int64)  # real edges only
    order = np.argsort(dst, kind='stable')
    srcs_sorted = src[order].astype(np.int64)
    row_ptr = np.zeros(npad + 1, np.int64)
    np.cumsum(deg, out=row_ptr[1:])

    gorder = np.argsort(-deg, kind='stable')        # global degree sort
    r = np.arange(npad)
    w_of = r // P
    core_of = w_of % ncores
    loc_of = (w_of // ncores) * P + (r % P)
    pos = np.empty(npad, np.int64)                  # node -> local row
    pos[gorder] = core_of * nk + loc_of
    nodes_at = np.empty(npad, np.int64)             # local row -> node
    nodes_at[core_of * nk + loc_of] = gorder

    # shared-table layout is chunk-major so AllGather chunks are contiguous:
    # row = chunk_base*ncores + core*chunk_size + (loc - chunk_start)
    sup = max(s for s in range(1, 9) if nwin % s == 0)
    nsup = nwin // sup
    cb = (nsup // 2) * sup * P                      # chunk boundary (rows)
    in_c1 = loc_of >= cb
    pos_tab = np.where(
        in_c1,
        cb * ncores + core_of * (nk - cb) + (loc_of - cb),
        core_of * cb + loc_of)
    postab = np.empty(npad, np.int64)               # node -> shared-table row
    postab[gorder] = pos_tab

    deg_sorted = deg[gorder].reshape(gwin, P)
    k_gwin = deg_sorted.max(axis=1)                 # data-K per global window
    k_loc = k_gwin.reshape(nwin, ncores).T
    k_uni = k_loc.max(axis=0) + 1                   # +1 self column (slot 0)
    k_uni = np.maximum(k_uni, 2)

    sws = []
    w = 0
    colstart = 0
    while w < nwin:
        kg = int(k_uni[w])
        nw = 1
        while (w + nw < nwin and nw < wmax
               and (nw + 1) * max(kg, int(k_uni[w + nw])) <= colmax):
            kg = max(kg, int(k_uni[w + nw]))
            nw += 1
        sws.append((w, nw, kg, colstart))
        colstart += nw * kg
        w += nw
    totc = colstart

    offs = np.zeros((ncores, P, totc), np.int32)    # pad slots -> row 0
    mask = np.zeros((ncores, P, totc), np.float32)
    ar = np.arange(P)
    pos32 = postab.astype(np.int32)
    for c in range(ncores):
        for (ws, nw, kg, cs) in sws:
            kar = np.arange(kg - 1)
            for wl in range(nw):
                rows = c * nk + (ws + wl) * P + ar
                gids = nodes_at[rows]
                dg = deg[gids]
                st = row_ptr[gids]
                idx = st[:, None] + kar[None, :]
                valid = kar[None, :] < dg[:, None]
                sv = srcs_sorted[np.minimum(idx, max(len(srcs_sorted) - 1, 0))]
                c0 = cs + wl * kg
                offs[c, :, c0] = rows                       # self (not gathered)
                mask[c, :, c0] = 1.0
                offs[c, :, c0 + 1: c0 + kg] = np.where(valid, pos32[sv], 0)
                mask[c, :, c0 + 1: c0 + kg] = valid

    return dict(nk=nk, npad=npad, nwin=nwin, totc=totc, sws=sws,
                offs=offs, mask=mask, nodes_at=nodes_at, pos=pos,
                n_edges=len(srcs_sorted))


# ----------------------------------------------------------------------------
def _build_program(nk, nwin, totc, sws, ncores, in_ch=IN_CH, reps=1,
                   ablate=""):
    no_gather = "gather" in ablate
    no_coll = "coll" in ablate
    no_compute = "compute" in ablate
    npad = nk * ncores
    nhalf = in_ch // P

    nc = bacc.Bacc("TRN2")
    xt = nc.declare_dram_parameter("xt", [in_ch, nk], bf16, isOutput=False)
    w1e = nc.declare_dram_parameter("w1e", [in_ch, 80], bf16, isOutput=False)
    w2e = nc.declare_dram_parameter("w2e", [64, 66], f32, isOutput=False)
    b1r = nc.declare_dram_parameter("b1r", [P, 64], f32, isOutput=False)
    b2r = nc.declare_dram_parameter("b2r", [P, 64], f32, isOutput=False)
    offs = nc.declare_dram_parameter("offs", [P, totc], i32, isOutput=False)
    msk = nc.declare_dram_parameter("msk", [P, totc], bf16, isOutput=False)
    outp = nc.declare_dram_parameter("out", [nk, 64], f32, isOutput=True)

    g1loc = nc.dram_tensor("g1loc", [nk, D1], bf16)
    g2loc = nc.dram_tensor("g2loc", [nk, D1], bf16)
    g1 = nc.dram_tensor("g1", [npad, D1], bf16, addr_space="Shared")
    g2 = nc.dram_tensor("g2", [npad, D1], bf16, addr_space="Shared")
    rg = [list(range(ncores))]

    SUP = max(s for s in range(1, 9) if nwin % s == 0)
    nsup = nwin // SUP
    scb = nsup // 2                     # superchunks in chunk 0
    cb = scb * SUP * P                  # chunk boundary (matches _preprocess)

    with ExitStack() as ctx:
        tc = ctx.enter_context(tile.TileContext(nc))
        cp = ctx.enter_context(tc.tile_pool(name="const", bufs=1))
        sa = ctx.enter_context(tc.tile_pool(name="sa", bufs=2))
        sb = ctx.enter_context(tc.tile_pool(name="sb", bufs=2))
        sbw = ctx.enter_context(tc.tile_pool(name="sbw", bufs=2))
        sw1 = ctx.enter_context(tc.tile_pool(name="sw1", bufs=1))
        ps = ctx.enter_context(tc.tile_pool(name="ps", bufs=2, space="PSUM"))

        w1sb = []
        for h in range(nhalf):
            t = cp.tile([P, 80], bf16, tag=f"w1_{h}")
            nc.sync.dma_start(out=t[:], in_=w1e[h * P:(h + 1) * P, :])
            w1sb.append(t)
        w2sb = cp.tile([64, 66], f32)
        nc.sync.dma_start(out=w2sb[:], in_=w2e[:])
        identf = cp.tile([P, P], f32)
        make_identity(nc, identf[:])
        b1sb = cp.tile([P, 64], f32)
        nc.sync.dma_start(out=b1sb[:], in_=b1r[:])
        b2sb = cp.tile([P, 64], f32)
        nc.sync.dma_start(out=b2sb[:], in_=b2r[:])
        ad2all = cp.tile([P, nwin], f32)
        ad1all = cp.tile([P, nwin * 8], f32)
        offs_sb = cp.tile([P, totc], i32)
        nc.sync.dma_start(out=offs_sb[:], in_=offs[:])
        msk_sb = cp.tile([P, totc], bf16)
        nc.sync.dma_start(out=msk_sb[:], in_=msk[:])

        for _ in range(reps):
            # ---------------- phase A ----------------
            for sc in range(nsup):
                xts = []
                for h in range(nhalf):
                    t = sa.tile([P, SUP * P], bf16, tag=f"xt_{h}")
                    nc.sync.dma_start(
                        out=t[:],
                        in_=xt[h * P:(h + 1) * P,
                               sc * SUP * P:(sc + 1) * SUP * P])
                    xts.append(t)
                g1rows = sa.tile([P, SUP * D1], bf16, tag="g1rows")
                for wl in range(SUP):
                    t1_ps = ps.tile([P, 80], f32, tag="t1")
                    for h in range(nhalf):
                        nc.tensor.matmul(
                            out=t1_ps[:],
                            lhsT=xts[h][:, wl * P:(wl + 1) * P],
                            rhs=w1sb[h][:],
                            start=(h == 0), stop=(h == nhalf - 1))
                    nc.scalar.copy(out=g1rows[:, wl * D1:(wl + 1) * D1],
                                   in_=t1_ps[:, 0:D1])
                    cix = sc * SUP + wl
                    nc.vector.tensor_copy(
                        out=ad1all[:, cix * 8:(cix + 1) * 8],
                        in_=t1_ps[:, D1:80])
                nc.sync.dma_start(
                    out=g1loc[sc * SUP * P:(sc + 1) * SUP * P, :]
                        .rearrange("(w p) d -> p w d", p=P),
                    in_=g1rows[:].rearrange("p (w d) -> p w d", d=D1))
                if sc == scb:   # one superchunk after data-ready: no SEQ stall
                    nc.gpsimd.collective_compute(
                        "AllGather", mybir.AluOpType.bypass,
                        ins=[g1loc[0:cb]],
                        outs=[g1[0:cb * ncores]],
                        replica_groups=rg)
            nc.gpsimd.collective_compute(
                "AllGather", mybir.AluOpType.bypass,
                ins=[g1loc[cb:nk]],
                outs=[g1[cb * ncores:npad]],
                replica_groups=rg)

            # ---------------- phase B ----------------
            nb2 = 0
            for gi, (ws, nw, kg, cs) in enumerate(sws):
                C = nw * kg
                gb = sbw.tile([P, C * D1], bf16, tag="gb")
                for cj in range(C):
                    if cj % kg == 0:
                        continue
                    nc.gpsimd.indirect_dma_start(
                        out=gb[:, cj * D1:(cj + 1) * D1], out_offset=None,
                        in_=g1[:],
                        in_offset=bass.IndirectOffsetOnAxis(
                            ap=offs_sb[:, cs + cj:cs + cj + 1], axis=0))
                # self columns (k=0 of each window): one dense DMA
                nc.sync.dma_start(
                    out=gb[:].rearrange("p (w k d) -> p w k d", k=kg, d=D1)[
                        :, :, 0:1, :].squeeze(2),
                    in_=g1loc[ws * P:(ws + nw) * P, :]
                        .rearrange("(w p) d -> p w d", p=P))

                gb3 = gb[:].rearrange("p (c d) -> p c d", d=D1)
                alpha = sb.tile([P, C * 8], bf16, tag="alpha")
                a4 = alpha[:].rearrange("p (w k h) -> p w k h", k=kg, h=8)
                nc.vector.tensor_tensor(
                    out=a4,
                    in0=gb3[:, :, 64:72].rearrange("p (w k) h -> p w k h", k=kg),
                    in1=ad1all[:, ws * 8:(ws + nw) * 8]
                        .rearrange("p (w h) -> p w h", h=8)
                        .unsqueeze(2).to_broadcast([P, nw, kg, 8]),
                    op=mybir.AluOpType.add)
                lr = sb.tile([P, C * 8], bf16, tag="lr")
                nc.vector.tensor_scalar_mul(out=lr[:], in0=alpha[:],
                                            scalar1=SLOPE)
                nc.vector.tensor_tensor(out=lr[:], in0=alpha[:], in1=lr[:],
                                        op=mybir.AluOpType.max)
                u = lr
                nc.scalar.activation(out=u[:], in_=lr[:],
                                     func=mybir.ActivationFunctionType.Exp)
                u3 = u[:].rearrange("p (c h) -> p c h", h=8)
                nc.vector.tensor_tensor(
                    out=u3, in0=u3,
                    in1=msk_sb[:, cs:cs + C].unsqueeze(2).to_broadcast(
                        [P, C, 8]),
                    op=mybir.AluOpType.mult)
                wgh = sw1.tile([P, C * 64], bf16, tag="wgh")
                nc.vector.tensor_tensor(
                    out=wgh[:].rearrange("p (c h d) -> p c h d", h=8, d=8),
                    in0=gb3[:, :, 0:64].rearrange("p c (h d) -> p c h d", d=8),
                    in1=u3.unsqueeze(3).to_broadcast([P, C, 8, 8]),
                    op=mybir.AluOpType.mult)
                numer = sb.tile([P, nw * 64], f32, tag="numer")
                nc.vector.tensor_reduce(
                    out=numer[:].rearrange("p (w hc) -> p w hc", hc=64),
                    in_=wgh[:].rearrange("p (w k hc) -> p w hc k", k=kg, hc=64),
                    axis=mybir.AxisListType.X, op=mybir.AluOpType.add)
                denom = sb.tile([P, nw * 8], f32, tag="denom")
                nc.vector.tensor_reduce(
                    out=denom[:].rearrange("p (w h) -> p w h", h=8),
                    in_=u[:].rearrange("p (w k h) -> p w h k", k=kg, h=8),
                    axis=mybir.AxisListType.X, op=mybir.AluOpType.add)
                nc.vector.tensor_scalar_max(out=denom[:], in0=denom[:],
                                            scalar1=1e-30)
                recip = sb.tile([P, nw * 8], f32, tag="recip")
                nc.vector.reciprocal(out=recip[:], in_=denom[:])
                z = sb.tile([P, nw * 64], f32, tag="z")
                z4 = z[:].rearrange("p (w h d) -> p w h d", h=8, d=8)
                nc.vector.tensor_tensor(
                    out=z4,
                    in0=numer[:].rearrange("p (w h d) -> p w h d", h=8, d=8),
                    in1=recip[:].rearrange("p (w h) -> p w h", h=8)
                        .unsqueeze(3).to_broadcast([P, nw, 8, 8]),
                    op=mybir.AluOpType.mult)
                z3 = z[:].rearrange("p (w d) -> p w d", d=64)
                nc.vector.tensor_tensor(
                    out=z3, in0=z3,
                    in1=b1sb[:].unsqueeze(1).to_broadcast([P, nw, 64]),
                    op=mybir.AluOpType.add)
                zneg = sb.tile([P, nw * 64], f32, tag="zneg")
                nc.vector.tensor_scalar_min(out=zneg[:], in0=z[:], scalar1=0.0)
                nc.scalar.activation(out=zneg[:], in_=zneg[:],
                                     func=mybir.ActivationFunctionType.Exp)
                nc.vector.tensor_scalar_add(out=zneg[:], in0=zneg[:],
                                            scalar1=-1.0)
                nc.vector.tensor_scalar_max(out=z[:], in0=z[:], scalar1=0.0)
                nc.vector.tensor_tensor(out=z[:], in0=z[:], in1=zneg[:],
                                        op=mybir.AluOpType.add)
                g2rows = sb.tile([P, nw * D1], bf16, tag="g2rows")
                nc.vector.memset(g2rows[:], 0.0)
                for wl in range(nw):
                    zT_ps = ps.tile([64, P], f32, tag="zt")
                    nc.tensor.transpose(out=zT_ps[:],
                                        in_=z[:, wl * 64:(wl + 1) * 64],
                                        identity=identf[:])
                    zT = sb.tile([64, P], f32, tag="zts")
                    nc.vector.tensor_copy(out=zT[:], in_=zT_ps[:])
                    t2_ps = ps.tile([P, 66], f32, tag="t2")
                    nc.tensor.matmul(out=t2_ps[:], lhsT=zT[:], rhs=w2sb[:],
                                     start=True, stop=True)
                    nc.vector.tensor_copy(
                        out=g2rows[:, wl * D1:wl * D1 + 65],
                        in_=t2_ps[:, 0:65])
                    nc.scalar.copy(out=ad2all[:, ws + wl:ws + wl + 1],
                                   in_=t2_ps[:, 65:66])
                nc.sync.dma_start(
                    out=g2loc[ws * P:(ws + nw) * P, :]
                        .rearrange("(w p) d -> p w d", p=P),
                    in_=g2rows[:].rearrange("p (w d) -> p w d", d=D1))
                # AllGather g2 in the same 2 chunks as g1 (contiguous outs).
                # Chunk 0 is emitted 2 groups after its data is ready so the
                # in-order Pool queue never stalls waiting on stores.
                if nb2 == 0 and (ws + nw) * P >= cb:
                    nb2 = gi + 2
                if nb2 and gi == min(nb2, len(sws) - 1):
                    nc.gpsimd.collective_compute(
                        "AllGather", mybir.AluOpType.bypass,
                        ins=[g2loc[0:cb]],
                        outs=[g2[0:cb * ncores]],
                        replica_groups=rg)
                    nb2 = -len(sws)
                if gi == len(sws) - 1:
                    nc.gpsimd.collective_compute(
                        "AllGather", mybir.AluOpType.bypass,
                        ins=[g2loc[cb:nk]],
                        outs=[g2[cb * ncores:npad]],
                        replica_groups=rg)

            # ---------------- phase C ----------------
            for (ws, nw, kg, cs) in sws:
                C = nw * kg
                gb = sbw.tile([P, C * D1], bf16, tag="gb")
                for cj in range(C):
                    if cj % kg == 0:
                        continue
                    nc.gpsimd.indirect_dma_start(
                        out=gb[:, cj * D1:(cj + 1) * D1], out_offset=None,
                        in_=g2[:],
                        in_offset=bass.IndirectOffsetOnAxis(
                            ap=offs_sb[:, cs + cj:cs + cj + 1], axis=0))
                nc.sync.dma_start(
                    out=gb[:].rearrange("p (w k d) -> p w k d", k=kg, d=D1)[
                        :, :, 0:1, :].squeeze(2),
                    in_=g2loc[ws * P:(ws + nw) * P, :]
                        .rearrange("(w p) d -> p w d", p=P))
                gb3 = gb[:].rearrange("p (c d) -> p c d", d=D1)

                alpha = sb.tile([P, C], f32, tag="alpha2")
                a3 = alpha[:].rearrange("p (w k) -> p w k", k=kg)
                nc.vector.tensor_tensor(
                    out=a3,
                    in0=gb3[:, :, 64:65].squeeze(2)
                        .rearrange("p (w k) -> p w k", k=kg),
                    in1=ad2all[:, ws:ws + nw].unsqueeze(2)
                        .to_broadcast([P, nw, kg]),
                    op=mybir.AluOpType.add)
                lr = sb.tile([P, C], f32, tag="lr2")
                nc.vector.tensor_scalar_mul(out=lr[:], in0=alpha[:],
                                            scalar1=SLOPE)
                nc.vector.tensor_tensor(out=lr[:], in0=alpha[:], in1=lr[:],
                                        op=mybir.AluOpType.max)
                u = lr
                nc.scalar.activation(out=u[:], in_=lr[:],
                                     func=mybir.ActivationFunctionType.Exp)
                nc.vector.tensor_tensor(out=u[:], in0=u[:],
                                        in1=msk_sb[:, cs:cs + C],
                                        op=mybir.AluOpType.mult)
                wgh = sw1.tile([P, C * 64], bf16, tag="wgh")
                nc.vector.tensor_tensor(
                    out=wgh[:].rearrange("p (c d) -> p c d", d=64),
                    in0=gb3[:, :, 0:64],
                    in1=u[:].unsqueeze(2).to_broadcast([P, C, 64]),
                    op=mybir.AluOpType.mult)
                numer = sb.tile([P, nw * 64], f32, tag="numer")
                nc.vector.tensor_reduce(
                    out=numer[:].rearrange("p (w d) -> p w d", d=64),
                    in_=wgh[:].rearrange("p (w k d) -> p w d k", k=kg, d=64),
                    axis=mybir.AxisListType.X, op=mybir.AluOpType.add)
                denom = sb.tile([P, nw], f32, tag="denom2")
                nc.vector.tensor_reduce(
                    out=denom[:].unsqueeze(2).squeeze(2),
                    in_=u[:].rearrange("p (w k) -> p w k", k=kg),
                    axis=mybir.AxisListType.X, op=mybir.AluOpType.add)
                nc.vector.tensor_scalar_max(out=denom[:], in0=denom[:],
                                            scalar1=1e-30)
                recip = sb.tile([P, nw], f32, tag="recip2")
                nc.vector.reciprocal(out=recip[:], in_=denom[:])
                o2 = sb.tile([P, nw * 64], f32, tag="o2")
                o3 = o2[:].rearrange("p (w d) -> p w d", d=64)
                nc.vector.tensor_tensor(
                    out=o3,
                    in0=numer[:].rearrange("p (w d) -> p w d", d=64),
                    in1=recip[:].unsqueeze(2).to_broadcast([P, nw, 64]),
                    op=mybir.AluOpType.mult)
                nc.vector.tensor_tensor(
                    out=o3, in0=o3,
                    in1=b2sb[:].unsqueeze(1).to_broadcast([P, nw, 64]),
                    op=mybir.AluOpType.add)
                mx = sb.tile([P, nw], f32, tag="mx")
                nc.vector.tensor_reduce(
                    out=mx[:].unsqueeze(2).squeeze(2), in_=o3,
                    axis=mybir.AxisListType.X, op=mybir.AluOpType.max)
                nc.vector.tensor_tensor(
                    out=o3, in0=o3,
                    in1=mx[:].unsqueeze(2).to_broadcast([P, nw, 64]),
                    op=mybir.AluOpType.subtract)
                ex = sb.tile([P, nw * 64], f32, tag="ex")
                nc.scalar.activation(out=ex[:], in_=o2[:],
                                     func=mybir.ActivationFunctionType.Exp)
                se = sb.tile([P, nw], f32, tag="se")
                nc.vector.tensor_reduce(
                    out=se[:].unsqueeze(2).squeeze(2),
                    in_=ex[:].rearrange("p (w d) -> p w d", d=64),
                    axis=mybir.AxisListType.X, op=mybir.AluOpType.add)
                nc.scalar.activation(out=se[:], in_=se[:],
                                     func=mybir.ActivationFunctionType.Ln)
                nc.vector.tensor_tensor(
                    out=o3, in0=o3,
                    in1=se[:].unsqueeze(2).to_broadcast([P, nw, 64]),
                    op=mybir.AluOpType.subtract)
                nc.sync.dma_start(
                    out=outp[ws * P:(ws + nw) * P, :]
                        .rearrange("(w p) d -> p w d", p=P),
                    in_=o2[:].rearrange("p (w d) -> p w d", d=64))
    nc.compile()
    return nc


# ----------------------------------------------------------------------------
def _expand_weights(W1, att_src1, att_dst1, W2, att_src2, att_dst2):
    W1 = np.asarray(W1, np.float32)
    a1s = np.zeros((HEADS * HID, HEADS), np.float32)
    a1s[np.arange(HEADS * HID), np.arange(HEADS * HID) // HID] = \
        np.asarray(att_src1, np.float32).reshape(-1)
    a1d = np.zeros((HEADS * HID, HEADS), np.float32)
    a1d[np.arange(HEADS * HID), np.arange(HEADS * HID) // HID] = \
        np.asarray(att_dst1, np.float32).reshape(-1)
    w1e = np.concatenate([W1, W1 @ a1s, W1 @ a1d], axis=1)
    W2 = np.asarray(W2, np.float32)
    w2e = np.concatenate(
        [W2,
         W2 @ np.asarray(att_src2, np.float32).T,
         W2 @ np.asarray(att_dst2, np.float32).T], axis=1)
    return np.ascontiguousarray(w1e), np.ascontiguousarray(w2e)


def _make_in_maps(pre, x, w1e, w2e, b1, b2, ncores):
    import ml_dtypes
    nk = pre['nk']
    n = x.shape[0]
    npad = pre['npad']
    xpad = np.zeros((npad, x.shape[1]), np.float32)
    xpad[:n] = np.asarray(x, np.float32)
    xperm = xpad[pre['nodes_at']]                  # table-row order
    w1e_bf = w1e.astype(ml_dtypes.bfloat16)
    b1r = np.broadcast_to(np.asarray(b1, np.float32)[None, :], (P, 64)).copy()
    b2r = np.broadcast_to(np.asarray(b2, np.float32)[None, :], (P, 64)).copy()
    in_maps = []
    for c in range(ncores):
        xtc = np.ascontiguousarray(
            xperm[c * nk:(c + 1) * nk].T).astype(ml_dtypes.bfloat16)
        in_maps.append(dict(
            xt=xtc,
            w1e=w1e_bf, w2e=w2e, b1r=b1r, b2r=b2r,
            offs=np.ascontiguousarray(pre['offs'][c]),
            msk=np.ascontiguousarray(
                pre['mask'][c].astype(ml_dtypes.bfloat16)),
        ))
    return in_maps


def _postprocess(res, pre, n, ncores):
    out = np.concatenate([res[c]["out"] for c in range(ncores)], axis=0)
    return np.ascontiguousarray(out[pre['pos'][:n]]).astype(np.float32)


def kernel(x, edge_index, W1, att_src1, att_dst1, b1,
           W2, att_src2, att_dst2, b2):
    import os
    os.environ["BASS_NEVER_TRACE"] = "1"
    from concourse.bass_utils import run_bass_kernel_spmd
    ncores = 8
    n = x.shape[0]
    pre = _preprocess(np.asarray(edge_index), n, ncores)
    w1e, w2e = _expand_weights(W1, att_src1, att_dst1, W2, att_src2, att_dst2)
    in_maps = _make_in_maps(pre, x, w1e, w2e, b1, b2, ncores)
    nc = _build_program(pre['nk'], pre['nwin'], pre['totc'], pre['sws'],
                        ncores, in_ch=x.shape[1])
    res = run_bass_kernel_spmd(nc, in_maps, list(range(ncores))).results
    return _postprocess(res, pre, n, ncores)


# revision 33
# speedup vs baseline: 1.8872x; 1.4824x over previous
"""2-layer GAT (PyG semantics) on 8 Trainium2 NeuronCores.

Layout: global degree-sorted node placement, window-interleaved cores.

Node with global-degree-rank r sits in window w = r // 128, partition
p = r % 128; window w belongs to core w % 8 at local window wl = w // 8.
Node table row (in the AllGathered tables) = core * nk + wl * 128 + p.
All per-node tensors (a_dst tables, layer-2 table rows, output rows) are
stored window-major, so every store and every per-node load is a contiguous
DMA; only the per-edge source-row fetch is indirect.

k-slot 0 of every window is the self-loop: its source row is the node
itself, which lives at an affine position in the core's local table slice,
so it is loaded with one dense DMA per group instead of indirect gathers.
"""
import sys

sys.path.insert(0, '/opt/trn_rl_repo')

from contextlib import ExitStack

import numpy as np

import concourse.bass as bass
import concourse.bacc as bacc
import concourse.mybir as mybir
import concourse.tile as tile
from concourse.masks import make_identity

P = 128
SLOPE = 0.2
D1 = 72          # gather row: [h(64) | a_src(8)]
HEADS = 8
HID = 8
IN_CH = 256

f32 = mybir.dt.float32
bf16 = mybir.dt.bfloat16
i32 = mybir.dt.int32


# ----------------------------------------------------------------------------
def _preprocess(edge_index, n, ncores, colmax=64, wmax=4):
    src = np.asarray(edge_index[0], dtype=np.int64)
    dst = np.asarray(edge_index[1], dtype=np.int64)

    nk = -(-n // (ncores * P)) * P
    npad = nk * ncores
    nwin = nk // P          # local windows per core
    gwin = nwin * ncores    # global windows

    deg = np.bincount(dst, minlength=npad).astype(np.int64)  # real edges only
    order = np.argsort(dst, kind='stable')
    srcs_sorted = src[order].astype(np.int64)
    row_ptr = np.zeros(npad + 1, np.int64)
    np.cumsum(deg, out=row_ptr[1:])

    gorder = np.argsort(-deg, kind='stable')        # global degree sort
    r = np.arange(npad)
    w_of = r // P
    core_of = w_of % ncores
    loc_of = (w_of // ncores) * P + (r % P)
    pos = np.empty(npad, np.int64)                  # node -> local row
    pos[gorder] = core_of * nk + loc_of
    nodes_at = np.empty(npad, np.int64)             # local row -> node
    nodes_at[core_of * nk + loc_of] = gorder

    # shared-table layout is chunk-major so AllGather chunks are contiguous:
    # row = chunk_base*ncores + core*chunk_size + (loc - chunk_start)
    sup = max(s for s in range(1, 9) if nwin % s == 0)
    nsup = nwin // sup
    cb = (nsup // 2) * sup * P                      # chunk boundary (rows)
    in_c1 = loc_of >= cb
    pos_tab = np.where(
        in_c1,
        cb * ncores + core_of * (nk - cb) + (loc_of - cb),
        core_of * cb + loc_of)
    postab = np.empty(npad, np.int64)               # node -> shared-table row
    postab[gorder] = pos_tab

    deg_sorted = deg[gorder].reshape(gwin, P)
    k_gwin = deg_sorted.max(axis=1)                 # data-K per global window
    k_loc = k_gwin.reshape(nwin, ncores).T
    k_uni = k_loc.max(axis=0) + 1                   # +1 self column (slot 0)
    k_uni = np.maximum(k_uni, 2)

    sws = []
    w = 0
    colstart = 0
    while w < nwin:
        kg = int(k_uni[w])
        nw = 1
        while (w + nw < nwin and nw < wmax
               and (nw + 1) * max(kg, int(k_uni[w + nw])) <= colmax):
            kg = max(kg, int(k_uni[w + nw]))
            nw += 1
        sws.append((w, nw, kg, colstart))
        colstart += nw * kg
        w += nw
    totc = colstart

    offs = np.zeros((ncores, P, totc), np.int32)    # pad slots -> row 0
    mask = np.zeros((ncores, P, totc), np.float32)
    ar = np.arange(P)
    pos32 = postab.astype(np.int32)
    for c in range(ncores):
        for (ws, nw, kg, cs) in sws:
            kar = np.arange(kg - 1)
            for wl in range(nw):
                rows = c * nk + (ws + wl) * P + ar
                gids = nodes_at[rows]
                dg = deg[gids]
                st = row_ptr[gids]
                idx = st[:, None] + kar[None, :]
                valid = kar[None, :] < dg[:, None]
                sv = srcs_sorted[np.minimum(idx, max(len(srcs_sorted) - 1, 0))]
                c0 = cs + wl * kg
                offs[c, :, c0] = rows                       # self (not gathered)
                mask[c, :, c0] = 1.0
                offs[c, :, c0 + 1: c0 + kg] = np.where(valid, pos32[sv], 0)
                mask[c, :, c0 + 1: c0 + kg] = valid

    return dict(nk=nk, npad=npad, nwin=nwin, totc=totc, sws=sws,
                offs=offs, mask=mask, nodes_at=nodes_at, pos=pos,
                n_edges=len(srcs_sorted))


# ----------------------------------------------------------------------------
def _build_program(nk, nwin, totc, sws, ncores, in_ch=IN_CH, reps=1,
                   ablate=""):
    no_gather = "gather" in ablate
    no_coll = "coll" in ablate
    no_compute = "compute" in ablate
    npad = nk * ncores
    nhalf = in_ch // P

    nc = bacc.Bacc("TRN2")
    xt = nc.declare_dram_parameter("xt", [in_ch, nk], bf16, isOutput=False)
    w1e = nc.declare_dram_parameter("w1e", [in_ch, 80], bf16, isOutput=False)
    w2e = nc.declare_dram_parameter("w2e", [64, 66], f32, isOutput=False)
    b1r = nc.declare_dram_parameter("b1r", [P, 64], f32, isOutput=False)
    b2r = nc.declare_dram_parameter("b2r", [P, 64], f32, isOutput=False)
    offs = nc.declare_dram_parameter("offs", [P, totc], i32, isOutput=False)
    msk = nc.declare_dram_parameter("msk", [P, totc], bf16, isOutput=False)
    outp = nc.declare_dram_parameter("out", [nk, 64], f32, isOutput=True)

    g1loc = nc.dram_tensor("g1loc", [nk, D1], bf16)
    g2loc = nc.dram_tensor("g2loc", [nk, D1], bf16)
    g1 = nc.dram_tensor("g1", [npad, D1], bf16, addr_space="Shared")
    g2 = nc.dram_tensor("g2", [npad, D1], bf16, addr_space="Shared")
    rg = [list(range(ncores))]

    SUP = max(s for s in range(1, 9) if nwin % s == 0)
    nsup = nwin // SUP
    scb = nsup // 2                     # superchunks in chunk 0
    cb = scb * SUP * P                  # chunk boundary (matches _preprocess)

    with ExitStack() as ctx:
        tc = ctx.enter_context(tile.TileContext(nc))
        cp = ctx.enter_context(tc.tile_pool(name="const", bufs=1))
        sa = ctx.enter_context(tc.tile_pool(name="sa", bufs=3))
        sb = ctx.enter_context(tc.tile_pool(name="sb", bufs=2))
        sbw = ctx.enter_context(tc.tile_pool(name="sbw", bufs=3))
        sw1 = ctx.enter_context(tc.tile_pool(name="sw1", bufs=2))
        ps = ctx.enter_context(tc.tile_pool(name="ps", bufs=2, space="PSUM"))

        w1sb = []
        for h in range(nhalf):
            t = cp.tile([P, 80], bf16, tag=f"w1_{h}")
            nc.sync.dma_start(out=t[:], in_=w1e[h * P:(h + 1) * P, :])
            w1sb.append(t)
        w2sb = cp.tile([64, 66], f32)
        nc.sync.dma_start(out=w2sb[:], in_=w2e[:])
        identf = cp.tile([P, P], f32)
        make_identity(nc, identf[:])
        b1sb = cp.tile([P, 64], f32)
        nc.sync.dma_start(out=b1sb[:], in_=b1r[:])
        b2sb = cp.tile([P, 64], f32)
        nc.sync.dma_start(out=b2sb[:], in_=b2r[:])
        ad2all = cp.tile([P, nwin], f32)
        ad1all = cp.tile([P, nwin * 8], f32)
        offs_sb = cp.tile([P, totc], i32)
        nc.sync.dma_start(out=offs_sb[:], in_=offs[:])
        msk_sb = cp.tile([P, totc], bf16)
        nc.sync.dma_start(out=msk_sb[:], in_=msk[:])

        for _ in range(reps):
            # ---------------- phase A ----------------
            for sc in range(nsup):
                xts = []
                for h in range(nhalf):
                    t = sa.tile([P, SUP * P], bf16, tag=f"xt_{h}")
                    nc.sync.dma_start(
                        out=t[:],
                        in_=xt[h * P:(h + 1) * P,
                               sc * SUP * P:(sc + 1) * SUP * P])
                    xts.append(t)
                g1rows = sa.tile([P, SUP * D1], bf16, tag="g1rows")
                for wl in range(SUP):
                    t1_ps = ps.tile([P, 80], f32, tag="t1")
                    for h in range(nhalf):
                        nc.tensor.matmul(
                            out=t1_ps[:],
                            lhsT=xts[h][:, wl * P:(wl + 1) * P],
                            rhs=w1sb[h][:],
                            start=(h == 0), stop=(h == nhalf - 1))
                    nc.scalar.copy(out=g1rows[:, wl * D1:(wl + 1) * D1],
                                   in_=t1_ps[:, 0:D1])
                    cix = sc * SUP + wl
                    nc.vector.tensor_copy(
                        out=ad1all[:, cix * 8:(cix + 1) * 8],
                        in_=t1_ps[:, D1:80])
                nc.sync.dma_start(
                    out=g1loc[sc * SUP * P:(sc + 1) * SUP * P, :]
                        .rearrange("(w p) d -> p w d", p=P),
                    in_=g1rows[:].rearrange("p (w d) -> p w d", d=D1))
                if sc == scb and not no_coll:
                    nc.gpsimd.collective_compute(
                        "AllGather", mybir.AluOpType.bypass,
                        ins=[g1loc[0:cb]],
                        outs=[g1[0:cb * ncores]],
                        replica_groups=rg)
            if not no_coll:
                nc.gpsimd.collective_compute(
                    "AllGather", mybir.AluOpType.bypass,
                    ins=[g1loc[cb:nk]],
                    outs=[g1[cb * ncores:npad]],
                    replica_groups=rg)

            # ---------------- phase B ----------------
            nb2 = 0
            for gi, (ws, nw, kg, cs) in enumerate(sws):
                C = nw * kg
                gb = sbw.tile([P, C * D1], bf16, tag="gb")
                for cj in range(C):
                    if cj % kg == 0 or no_gather:
                        continue
                    nc.gpsimd.indirect_dma_start(
                        out=gb[:, cj * D1:(cj + 1) * D1], out_offset=None,
                        in_=g1[:],
                        in_offset=bass.IndirectOffsetOnAxis(
                            ap=offs_sb[:, cs + cj:cs + cj + 1], axis=0))
                # self columns (k=0 of each window): one dense DMA
                nc.sync.dma_start(
                    out=gb[:].rearrange("p (w k d) -> p w k d", k=kg, d=D1)[
                        :, :, 0:1, :].squeeze(2),
                    in_=g1loc[ws * P:(ws + nw) * P, :]
                        .rearrange("(w p) d -> p w d", p=P))

                if nb2 == 0 and (ws + nw) * P >= cb:
                    nb2 = gi + 2
                if (nb2 > 0 and gi == min(nb2, len(sws) - 1)
                        and not no_coll):
                    nc.gpsimd.collective_compute(
                        "AllGather", mybir.AluOpType.bypass,
                        ins=[g2loc[0:cb]],
                        outs=[g2[0:cb * ncores]],
                        replica_groups=rg)
                    nb2 = -len(sws)
                if gi == len(sws) - 1 and not no_coll:
                    nc.gpsimd.collective_compute(
                        "AllGather", mybir.AluOpType.bypass,
                        ins=[g2loc[cb:nk]],
                        outs=[g2[cb * ncores:npad]],
                        replica_groups=rg)
                if no_compute:
                    continue

                gb3 = gb[:].rearrange("p (c d) -> p c d", d=D1)
                alpha = sb.tile([P, C * 8], bf16, tag="alpha")
                a4 = alpha[:].rearrange("p (w k h) -> p w k h", k=kg, h=8)
                nc.vector.tensor_tensor(
                    out=a4,
                    in0=gb3[:, :, 64:72].rearrange("p (w k) h -> p w k h", k=kg),
                    in1=ad1all[:, ws * 8:(ws + nw) * 8]
                        .rearrange("p (w h) -> p w h", h=8)
                        .unsqueeze(2).to_broadcast([P, nw, kg, 8]),
                    op=mybir.AluOpType.add)
                lr = sb.tile([P, C * 8], bf16, tag="lr")
                nc.vector.tensor_scalar_mul(out=lr[:], in0=alpha[:],
                                            scalar1=SLOPE)
                nc.vector.tensor_tensor(out=lr[:], in0=alpha[:], in1=lr[:],
                                        op=mybir.AluOpType.max)
                u = lr
                nc.scalar.activation(out=u[:], in_=lr[:],
                                     func=mybir.ActivationFunctionType.Exp)
                u3 = u[:].rearrange("p (c h) -> p c h", h=8)
                nc.vector.tensor_tensor(
                    out=u3, in0=u3,
                    in1=msk_sb[:, cs:cs + C].unsqueeze(2).to_broadcast(
                        [P, C, 8]),
                    op=mybir.AluOpType.mult)
                wgh = sw1.tile([P, C * 64], bf16, tag="wgh")
                nc.vector.tensor_tensor(
                    out=wgh[:].rearrange("p (c h d) -> p c h d", h=8, d=8),
                    in0=gb3[:, :, 0:64].rearrange("p c (h d) -> p c h d", d=8),
                    in1=u3.unsqueeze(3).to_broadcast([P, C, 8, 8]),
                    op=mybir.AluOpType.mult)
                numer = sb.tile([P, nw * 64], f32, tag="numer")
                nc.vector.tensor_reduce(
                    out=numer[:].rearrange("p (w hc) -> p w hc", hc=64),
                    in_=wgh[:].rearrange("p (w k hc) -> p w hc k", k=kg, hc=64),
                    axis=mybir.AxisListType.X, op=mybir.AluOpType.add)
                denom = sb.tile([P, nw * 8], f32, tag="denom")
                nc.vector.tensor_reduce(
                    out=denom[:].rearrange("p (w h) -> p w h", h=8),
                    in_=u[:].rearrange("p (w k h) -> p w h k", k=kg, h=8),
                    axis=mybir.AxisListType.X, op=mybir.AluOpType.add)
                nc.vector.tensor_scalar_max(out=denom[:], in0=denom[:],
                                            scalar1=1e-30)
                recip = sb.tile([P, nw * 8], f32, tag="recip")
                nc.vector.reciprocal(out=recip[:], in_=denom[:])
                z = sb.tile([P, nw * 64], f32, tag="z")
                z4 = z[:].rearrange("p (w h d) -> p w h d", h=8, d=8)
                nc.vector.tensor_tensor(
                    out=z4,
                    in0=numer[:].rearrange("p (w h d) -> p w h d", h=8, d=8),
                    in1=recip[:].rearrange("p (w h) -> p w h", h=8)
                        .unsqueeze(3).to_broadcast([P, nw, 8, 8]),
                    op=mybir.AluOpType.mult)
                z3 = z[:].rearrange("p (w d) -> p w d", d=64)
                nc.vector.tensor_tensor(
                    out=z3, in0=z3,
                    in1=b1sb[:].unsqueeze(1).to_broadcast([P, nw, 64]),
                    op=mybir.AluOpType.add)
                zneg = sb.tile([P, nw * 64], f32, tag="zneg")
                nc.vector.tensor_scalar_min(out=zneg[:], in0=z[:], scalar1=0.0)
                nc.scalar.activation(out=zneg[:], in_=zneg[:],
                                     func=mybir.ActivationFunctionType.Exp)
                nc.vector.tensor_scalar_add(out=zneg[:], in0=zneg[:],
                                            scalar1=-1.0)
                nc.vector.tensor_scalar_max(out=z[:], in0=z[:], scalar1=0.0)
                nc.vector.tensor_tensor(out=z[:], in0=z[:], in1=zneg[:],
                                        op=mybir.AluOpType.add)
                g2rows = sb.tile([P, nw * D1], bf16, tag="g2rows")
                nc.vector.memset(g2rows[:], 0.0)
                for wl in range(nw):
                    zT_ps = ps.tile([64, P], f32, tag="zt")
                    nc.tensor.transpose(out=zT_ps[:],
                                        in_=z[:, wl * 64:(wl + 1) * 64],
                                        identity=identf[:])
                    zT = sb.tile([64, P], f32, tag="zts")
                    nc.vector.tensor_copy(out=zT[:], in_=zT_ps[:])
                    t2_ps = ps.tile([P, 66], f32, tag="t2")
                    nc.tensor.matmul(out=t2_ps[:], lhsT=zT[:], rhs=w2sb[:],
                                     start=True, stop=True)
                    nc.vector.tensor_copy(
                        out=g2rows[:, wl * D1:wl * D1 + 65],
                        in_=t2_ps[:, 0:65])
                    nc.scalar.copy(out=ad2all[:, ws + wl:ws + wl + 1],
                                   in_=t2_ps[:, 65:66])
                nc.sync.dma_start(
                    out=g2loc[ws * P:(ws + nw) * P, :]
                        .rearrange("(w p) d -> p w d", p=P),
                    in_=g2rows[:].rearrange("p (w d) -> p w d", d=D1))

            # ---------------- phase C ----------------
            for (ws, nw, kg, cs) in sws:
                C = nw * kg
                gb = sbw.tile([P, C * D1], bf16, tag="gb")
                for cj in range(C):
                    if cj % kg == 0 or no_gather:
                        continue
                    nc.gpsimd.indirect_dma_start(
                        out=gb[:, cj * D1:(cj + 1) * D1], out_offset=None,
                        in_=g2[:],
                        in_offset=bass.IndirectOffsetOnAxis(
                            ap=offs_sb[:, cs + cj:cs + cj + 1], axis=0))
                nc.sync.dma_start(
                    out=gb[:].rearrange("p (w k d) -> p w k d", k=kg, d=D1)[
                        :, :, 0:1, :].squeeze(2),
                    in_=g2loc[ws * P:(ws + nw) * P, :]
                        .rearrange("(w p) d -> p w d", p=P))
                if no_compute:
                    continue
                gb3 = gb[:].rearrange("p (c d) -> p c d", d=D1)

                alpha = sb.tile([P, C], f32, tag="alpha2")
                a3 = alpha[:].rearrange("p (w k) -> p w k", k=kg)
                nc.vector.tensor_tensor(
                    out=a3,
                    in0=gb3[:, :, 64:65].squeeze(2)
                        .rearrange("p (w k) -> p w k", k=kg),
                    in1=ad2all[:, ws:ws + nw].unsqueeze(2)
                        .to_broadcast([P, nw, kg]),
                    op=mybir.AluOpType.add)
                lr = sb.tile([P, C], f32, tag="lr2")
                nc.vector.tensor_scalar_mul(out=lr[:], in0=alpha[:],
                                            scalar1=SLOPE)
                nc.vector.tensor_tensor(out=lr[:], in0=alpha[:], in1=lr[:],
                                        op=mybir.AluOpType.max)
                u = lr
                nc.scalar.activation(out=u[:], in_=lr[:],
                                     func=mybir.ActivationFunctionType.Exp)
                nc.vector.tensor_tensor(out=u[:], in0=u[:],
                                        in1=msk_sb[:, cs:cs + C],
                                        op=mybir.AluOpType.mult)
                wgh = sw1.tile([P, C * 64], bf16, tag="wgh")
                nc.vector.tensor_tensor(
                    out=wgh[:].rearrange("p (c d) -> p c d", d=64),
                    in0=gb3[:, :, 0:64],
                    in1=u[:].unsqueeze(2).to_broadcast([P, C, 64]),
                    op=mybir.AluOpType.mult)
                numer = sb.tile([P, nw * 64], f32, tag="numer")
                nc.vector.tensor_reduce(
                    out=numer[:].rearrange("p (w d) -> p w d", d=64),
                    in_=wgh[:].rearrange("p (w k d) -> p w d k", k=kg, d=64),
                    axis=mybir.AxisListType.X, op=mybir.AluOpType.add)
                denom = sb.tile([P, nw], f32, tag="denom2")
                nc.vector.tensor_reduce(
                    out=denom[:].unsqueeze(2).squeeze(2),
                    in_=u[:].rearrange("p (w k) -> p w k", k=kg),
                    axis=mybir.AxisListType.X, op=mybir.AluOpType.add)
                nc.vector.tensor_scalar_max(out=denom[:], in0=denom[:],
                                            scalar1=1e-30)
                recip = sb.tile([P, nw], f32, tag="recip2")
                nc.vector.reciprocal(out=recip[:], in_=denom[:])
                o2 = sb.tile([P, nw * 64], f32, tag="o2")
                o3 = o2[:].rearrange("p (w d) -> p w d", d=64)
                nc.vector.tensor_tensor(
                    out=o3,
                    in0=numer[:].rearrange("p (w d) -> p w d", d=64),
                    in1=recip[:].unsqueeze(2).to_broadcast([P, nw, 64]),
                    op=mybir.AluOpType.mult)
                nc.vector.tensor_tensor(
                    out=o3, in0=o3,
                    in1=b2sb[:].unsqueeze(1).to_broadcast([P, nw, 64]),
                    op=mybir.AluOpType.add)
                mx = sb.tile([P, nw], f32, tag="mx")
                nc.vector.tensor_reduce(
                    out=mx[:].unsqueeze(2).squeeze(2), in_=o3,
                    axis=mybir.AxisListType.X, op=mybir.AluOpType.max)
                nc.vector.tensor_tensor(
                    out=o3, in0=o3,
                    in1=mx[:].unsqueeze(2).to_broadcast([P, nw, 64]),
                    op=mybir.AluOpType.subtract)
                ex = sb.tile([P, nw * 64], f32, tag="ex")
                nc.scalar.activation(out=ex[:], in_=o2[:],
                                     func=mybir.ActivationFunctionType.Exp)
                se = sb.tile([P, nw], f32, tag="se")
                nc.vector.tensor_reduce(
                    out=se[:].unsqueeze(2).squeeze(2),
                    in_=ex[:].rearrange("p (w d) -> p w d", d=64),
                    axis=mybir.AxisListType.X, op=mybir.AluOpType.add)
                nc.scalar.activation(out=se[:], in_=se[:],
                                     func=mybir.ActivationFunctionType.Ln)
                nc.vector.tensor_tensor(
                    out=o3, in0=o3,
                    in1=se[:].unsqueeze(2).to_broadcast([P, nw, 64]),
                    op=mybir.AluOpType.subtract)
                nc.sync.dma_start(
                    out=outp[ws * P:(ws + nw) * P, :]
                        .rearrange("(w p) d -> p w d", p=P),
                    in_=o2[:].rearrange("p (w d) -> p w d", d=64))
    nc.compile()
    return nc


# ----------------------------------------------------------------------------
def _expand_weights(W1, att_src1, att_dst1, W2, att_src2, att_dst2):
    W1 = np.asarray(W1, np.float32)
    a1s = np.zeros((HEADS * HID, HEADS), np.float32)
    a1s[np.arange(HEADS * HID), np.arange(HEADS * HID) // HID] = \
        np.asarray(att_src1, np.float32).reshape(-1)
    a1d = np.zeros((HEADS * HID, HEADS), np.float32)
    a1d[np.arange(HEADS * HID), np.arange(HEADS * HID) // HID] = \
        np.asarray(att_dst1, np.float32).reshape(-1)
    w1e = np.concatenate([W1, W1 @ a1s, W1 @ a1d], axis=1)
    W2 = np.asarray(W2, np.float32)
    w2e = np.concatenate(
        [W2,
         W2 @ np.asarray(att_src2, np.float32).T,
         W2 @ np.asarray(att_dst2, np.float32).T], axis=1)
    return np.ascontiguousarray(w1e), np.ascontiguousarray(w2e)


def _make_in_maps(pre, x, w1e, w2e, b1, b2, ncores):
    import ml_dtypes
    nk = pre['nk']
    n = x.shape[0]
    npad = pre['npad']
    xpad = np.zeros((npad, x.shape[1]), np.float32)
    xpad[:n] = np.asarray(x, np.float32)
    xperm = xpad[pre['nodes_at']]                  # table-row order
    w1e_bf = w1e.astype(ml_dtypes.bfloat16)
    b1r = np.broadcast_to(np.asarray(b1, np.float32)[None, :], (P, 64)).copy()
    b2r = np.broadcast_to(np.asarray(b2, np.float32)[None, :], (P, 64)).copy()
    in_maps = []
    for c in range(ncores):
        xtc = np.ascontiguousarray(
            xperm[c * nk:(c + 1) * nk].T).astype(ml_dtypes.bfloat16)
        in_maps.append(dict(
            xt=xtc,
            w1e=w1e_bf, w2e=w2e, b1r=b1r, b2r=b2r,
            offs=np.ascontiguousarray(pre['offs'][c]),
            msk=np.ascontiguousarray(
                pre['mask'][c].astype(ml_dtypes.bfloat16)),
        ))
    return in_maps


def _postprocess(res, pre, n, ncores):
    out = np.concatenate([res[c]["out"] for c in range(ncores)], axis=0)
    return np.ascontiguousarray(out[pre['pos'][:n]]).astype(np.float32)


def kernel(x, edge_index, W1, att_src1, att_dst1, b1,
           W2, att_src2, att_dst2, b2):
    import os
    os.environ["BASS_NEVER_TRACE"] = "1"
    from concourse.bass_utils import run_bass_kernel_spmd
    ncores = 8
    n = x.shape[0]
    pre = _preprocess(np.asarray(edge_index), n, ncores)
    w1e, w2e = _expand_weights(W1, att_src1, att_dst1, W2, att_src2, att_dst2)
    in_maps = _make_in_maps(pre, x, w1e, w2e, b1, b2, ncores)
    nc = _build_program(pre['nk'], pre['nwin'], pre['totc'], pre['sws'],
                        ncores, in_ch=x.shape[1])
    res = run_bass_kernel_spmd(nc, in_maps, list(range(ncores))).results
    return _postprocess(res, pre, n, ncores)
